# revision 1
# baseline (speedup 1.0000x reference)
"""Allegro GNN on 8 TRN2 NeuronCores — Bass/Tile kernel.

Sharding: nodes are partitioned across cores (256 nodes/core); every edge is
routed to the core owning its *sender* node, so the scatter-sum over senders
and the gather-back are both core-local: zero collectives.

Per-core device program (feature-major activations, one-hot matmuls for
scatter/gather, tensor product via pairwise-product matmuls contracted by a
host-folded Clebsch-Gordan x Wv matrix):
  geometry (edge-major) -> two-body MLP -> L1 scatter/gather ->
  P=wYv(x)Y products -> C-matmul (Vn'+tp0) -> L1 MLP -> L2 scatter/gather ->
  wY2*Vn' dots -> L2 MLP -> head.
"""
import math
import sys

import numpy as np

sys.path.insert(0, "/opt/trn_rl_repo")

import concourse.bacc as bacc  # noqa: E402
import concourse.bass as bass  # noqa: E402
import concourse.mybir as mybir  # noqa: E402
from concourse import tile  # noqa: E402
from concourse.bass_utils import run_bass_kernel_spmd  # noqa: E402
import ml_dtypes  # noqa: E402

F32 = mybir.dt.float32
BF16 = mybir.dt.bfloat16
BF = ml_dtypes.bfloat16
AL = mybir.AluOpType
AF = mybir.ActivationFunctionType

E, NNODE = 32768, 2048
NUM_SPECIES, EMB = 100, 32
MUL, HIDDEN, N_RBF, LMAX = 16, 256, 8, 3
N_CORES = 8
NPC = NNODE // N_CORES          # nodes per core
SLSTART = {0: 0, 1: 1, 2: 4, 3: 9}

# ---------------------------------------------------------------- CG tensors


def _cg(j1, m1, j2, m2, j3, m3):
    if m1 + m2 != m3:
        return 0.0
    f = math.factorial
    pre = math.sqrt((2*j3+1) * f(j1+j2-j3) * f(j1-j2+j3) * f(-j1+j2+j3) / f(j1+j2+j3+1))
    pre *= math.sqrt(f(j3+m3)*f(j3-m3)*f(j1-m1)*f(j1+m1)*f(j2-m2)*f(j2+m2))
    s = 0.0
    kmin = max(0, j2 - j3 - m1, j1 - j3 + m2)
    kmax = min(j1 + j2 - j3, j1 - m1, j2 + m2)
    for k in range(kmin, kmax + 1):
        s += (-1)**k / (f(k)*f(j1+j2-j3-k)*f(j1-m1-k)*f(j2+m2-k)*f(j3-j2+m1+k)*f(j3-j1-m2+k))
    return pre * s


def _umat(l):
    U = np.zeros((2*l+1, 2*l+1), dtype=complex)
    U[l, l] = 1.0
    s2 = 1.0 / math.sqrt(2.0)
    for m in range(1, l + 1):
        U[l+m, l-m] = s2
        U[l+m, l+m] = (-1)**m * s2
        U[l-m, l-m] = 1j * s2
        U[l-m, l+m] = -1j * (-1)**m * s2
    return U


def _real_coupling(l1, l2, l3):
    C = np.zeros((2*l1+1, 2*l2+1, 2*l3+1), dtype=complex)
    for a, m1 in enumerate(range(-l1, l1+1)):
        for b, m2 in enumerate(range(-l2, l2+1)):
            for c, m3 in enumerate(range(-l3, l3+1)):
                C[a, b, c] = _cg(l1, m1, l2, m2, l3, m3)
    T = np.einsum('am,bn,ck,mnk->abc', _umat(l1), _umat(l2), _umat(l3).conj(), C)
    Tr, Ti = np.real(T), np.imag(T)
    T = Tr if np.linalg.norm(Tr) >= np.linalg.norm(Ti) else Ti
    n = np.linalg.norm(T)
    return None if n < 1e-8 else (T / n).astype(np.float32)


PATHS = {l3: [] for l3 in range(LMAX + 1)}
for _l1 in range(LMAX + 1):
    for _l2 in range(LMAX + 1):
        for _l3 in range(abs(_l1 - _l2), min(_l1 + _l2, LMAX) + 1):
            _T = _real_coupling(_l1, _l2, _l3)
            if _T is not None:
                PATHS[_l3].append((_l1, _l2, _T))
NPATH = {l3: len(PATHS[l3]) for l3 in range(LMAX + 1)}

# (p,k)-row table for the 160-row ttT / WK matrices: l3 1..3 then l3=0
_PKROWS = []
for _l3 in (1, 2, 3):
    for _p in range(NPATH[_l3]):
        for _kk in range(2*_l3+1):
            _PKROWS.append((_l3, _p, _kk))
_PK0_OFF = len(_PKROWS)                      # 152
for _p in range(NPATH[0]):
    _PKROWS.append((0, _p, 0))
NPK = len(_PKROWS)                           # 156
assert NPK == 156

# C output column layout: [0..63] tp0 (p*16+m), [64..303] Vn' (64 + d*15 + k-1)
NCOL = 304


def _build_ttT():
    """ttT[(p,k)-row, (i,j)] with scale_l3, fan, and L2-dot fold baked in."""
    # L2 fold: T2_l[k,k'] for the (l,l,0) paths
    T2 = {}
    for p, (l1, l2, T) in enumerate(PATHS[0]):
        T2[l1] = T[:, :, 0]
    tt = np.zeros((160, 256), np.float32)
    for r, (l3, p, ko) in enumerate(_PKROWS):
        if l3 == 0:
            l1, l2, T = PATHS[0][p]
            i0, j0 = SLSTART[l1], SLSTART[l2]
            for ii in range(2*l1+1):
                for jj in range(2*l2+1):
                    tt[152 + p, (i0+ii)*16 + (j0+jj)] = T[ii, jj, 0]
        else:
            l1, l2, T = PATHS[l3][p]
            i0, j0 = SLSTART[l1], SLSTART[l2]
            scale = math.sqrt(2*l3+1) / math.sqrt(MUL * NPATH[l3])
            # row (p, ko) carries sum_kk T2[ko,kk] * T[:,:,kk] * scale
            acc = np.zeros((2*l1+1, 2*l2+1), np.float64)
            for kk in range(2*l3+1):
                t2 = T2[l3][ko, kk]
                if t2 != 0.0:
                    acc += t2 * T[:, :, kk]
            tt[r, :] = 0.0
            blk = (acc * scale).astype(np.float32)
            for ii in range(2*l1+1):
                for jj in range(2*l2+1):
                    tt[r, (i0+ii)*16 + (j0+jj)] = blk[ii, jj]
    return tt


def _build_wk(W_v1, W_v2, W_v3):
    """wk[m, 160, 384]: pure placement of Wv values (+1s for tp0 rows)."""
    Wv = {1: W_v1, 2: W_v2, 3: W_v3}
    wk = np.zeros((MUL, 160, NCOL), np.float32)
    for r, (l3, p, ko) in enumerate(_PKROWS):
        if l3 == 0:
            for m in range(MUL):
                wk[m, 152 + p, p*16 + m] = 1.0
        else:
            kabs = SLSTART[l3] + ko            # 1..15
            for m in range(MUL):
                # cols (d, k=kabs) = 128 + d*15 + (kabs-1)
                wk[m, r, 64 + np.arange(MUL)*15 + (kabs-1)] = Wv[l3][p*16 + m, :]
    return wk


# ------------------------------------------------------------- device program

_PROG_CACHE = {}


def _build_program(CAP):
    NT = CAP // 128
    CH = [(s, min(512, CAP - s)) for s in range(0, CAP, 512)]
    nc = bacc.Bacc("TRN2", target_bir_lowering=False, debug=False,
                   num_devices=N_CORES)
    D = {}

    def dp(name, shape, dt=F32, out=False):
        D[name] = nc.declare_dram_parameter(name, list(shape), dt, isOutput=out)
        return D[name]

    dp("vec", [128, NT, 3]); dp("maskt", [128, NT])
    dp("ohs", [128, CAP], BF16); dp("ohr", [128, CAP], BF16)
    dp("smat", [NT, 128, 256], BF16); dp("gmat", [128, 2, CAP], BF16)
    dp("tabs", [128, 32], BF16); dp("tabr", [128, 32], BF16)
    dp("w1b", [8, 32], BF16)
    dp("wtb2", [32, 64], BF16); dp("wtb3", [64, 128], BF16); dp("wtb4", [128, 256], BF16)
    dp("ww0", [128, 2, 16], BF16); dp("ww1", [128, 2, 16], BF16); dp("ww2", [128, 2, 16], BF16)
    dp("wl11", [128, 2, 256], BF16); dp("wl11t", [64, 256], BF16)
    dp("wl12", [128, 2, 256], BF16); dp("wl13", [128, 2, 256], BF16)
    dp("wl21", [128, 2, 256], BF16); dp("wl21t", [48, 256], BF16)
    dp("wl22", [128, 2, 256], BF16); dp("wl23", [128, 2, 256], BF16)
    dp("wh", [128, 2, 128], BF16); dp("wout", [128, 1], BF16)
    dp("ttT", [128, 2, 256], BF16); dp("wk", [16, 160, NCOL], BF16)
    dp("repj", [16, 256], BF16); dp("repibig", [128, 8, 256], BF16)
    dp("e16b", [16, 256], BF16)
    dp("iden", [128, 128]); dp("idenb", [128, 128], BF16)
    dp("idenb64", [128, 64], BF16)
    dp("ones1", [1, 128], BF16)
    dp("outv", [1, CAP], out=True)

    S3 = math.sqrt(3.0); S15 = math.sqrt(15.0); S5 = math.sqrt(5.0)
    S358 = math.sqrt(35.0/8.0); S105 = math.sqrt(105.0)
    S218 = math.sqrt(21.0/8.0); S7 = math.sqrt(7.0)

    with tile.TileContext(nc) as tc:
        with tc.tile_pool(name="perm", bufs=1) as perm, \
             tc.tile_pool(name="wpool", bufs=1) as wpool, \
             tc.tile_pool(name="tmp", bufs=2) as tmp, \
             tc.tile_pool(name="chp", bufs=2) as chp, \
             tc.tile_pool(name="hp", bufs=1) as hp, \
             tc.tile_pool(name="pst", bufs=3, space="PSUM") as pst, \
             tc.tile_pool(name="psacc", bufs=1, space="PSUM") as psacc, \
             tc.tile_pool(name="psr", bufs=2, space="PSUM") as psr:

            # ---- persistent SBUF
            geo = perm.tile([128, NT, 32], F32, tag="geo", name="geo")
            ybf = perm.tile([16, CAP], BF16, tag="ybf", name="ybf")
            bbf = perm.tile([8, CAP], BF16, tag="bbf", name="bbf")
            envbf = perm.tile([1, CAP], BF16, tag="envbf", name="envbf")
            xsb = perm.tile([128, 2, CAP], BF16, tag="xsb", name="xsb")
            v16 = perm.tile([16, CAP], BF16, tag="v16", name="v16")
            tp02 = perm.tile([48, CAP], BF16, tag="tp02", name="tp02")
            vnpA = perm.tile([128, CAP], BF16, tag="vnpA", name="vnpA")
            vnpB = perm.tile([128, CAP], BF16, tag="vnpB", name="vnpB")
            vnpC = perm.tile([48, CAP], BF16, tag="vnpC", name="vnpC")
            cst = perm.tile([128, 2, 16, NCOL], BF16, tag="cst", name="cst")
            node_nm = perm.tile([128, 2, 256], BF16, tag="node_nm", name="node_nm")

            # ---- weights in SBUF
            W = {}
            for nm, shape, dt in [
                ("tabs", [128, 32], BF16), ("tabr", [128, 32], BF16),
                ("w1b", [8, 32], BF16),
                ("wtb2", [32, 64], BF16), ("wtb3", [64, 128], BF16),
                ("wtb4", [128, 256], BF16),
                ("ww0", [128, 2, 16], BF16), ("ww1", [128, 2, 16], BF16),
                ("ww2", [128, 2, 16], BF16),
                ("wl11", [128, 2, 256], BF16), ("wl11t", [64, 256], BF16),
                ("wl12", [128, 2, 256], BF16), ("wl13", [128, 2, 256], BF16),
                ("wl21", [128, 2, 256], BF16), ("wl21t", [48, 256], BF16),
                ("wl22", [128, 2, 256], BF16), ("wl23", [128, 2, 256], BF16),
                ("wh", [128, 2, 128], BF16), ("wout", [128, 1], BF16),
                ("ttT", [128, 2, 256], BF16),
                ("repj", [16, 256], BF16), ("repibig", [128, 8, 256], BF16),
                ("e16b", [16, 256], BF16),
                ("iden", [128, 128], F32), ("idenb", [128, 128], BF16),
                ("idenb64", [128, 64], BF16),
                ("ones1", [1, 128], BF16),
            ]:
                W[nm] = wpool.tile(shape, dt, tag="w_" + nm, name="w_" + nm)
                nc.sync.dma_start(W[nm][:], D[nm][:])

            # ================= C build: Cst[:, oc, m, :] = ttT.T @ wk[m]
            for m in range(MUL):
                wkt = tmp.tile([128, 2 * NCOL], BF16, tag="wkt", name="wkt")
                nc.sync.dma_start(wkt[:, 0:NCOL], D["wk"][m, 0:128, :])
                nc.sync.dma_start(wkt[0:32, NCOL:2*NCOL], D["wk"][m, 128:160, :])
                for oc in range(2):
                    psc = pst.tile([128, NCOL], F32, tag="ps", name="psc")
                    nc.tensor.matmul(psc[:], W["ttT"][:, 0, oc*128:(oc+1)*128],
                                     wkt[:, 0:NCOL], start=True, stop=False)
                    nc.tensor.matmul(psc[:], W["ttT"][0:32, 1, oc*128:(oc+1)*128],
                                     wkt[0:32, NCOL:2*NCOL], start=False, stop=True)
                    nc.vector.tensor_copy(cst[:, oc, m, :], psc[:])

            # ================= geometry (edge-major)
            vec = perm.tile([128, NT, 3], F32, tag="vec", name="vec")
            u = perm.tile([128, NT, 3], F32, tag="u", name="u")
            nc.sync.dma_start(vec[:], D["vec"][:])
            mask = tmp.tile([128, NT], F32, tag="mask", name="mask")
            nc.sync.dma_start(mask[:], D["maskt"][:])

            def t2(tag):
                return tmp.tile([128, NT], F32, tag=tag, name=tag)

            vv = tmp.tile([128, NT, 3], F32, tag="vv", name="vv")
            nc.vector.tensor_tensor(vv[:], vec[:], vec[:], op=AL.mult)
            d2 = t2("d2")
            nc.vector.tensor_reduce(d2[:], vv[:], axis=mybir.AxisListType.X, op=AL.add)
            d = t2("d")
            nc.scalar.activation(d[:], d2[:], AF.Sqrt)
            rec = t2("rec")
            nc.vector.reciprocal(rec[:], d[:])
            nc.vector.tensor_tensor(
                u[:], vec[:], rec[:, :, None].broadcast_to((128, NT, 3)), op=AL.mult)
            # envelope (p=6): 1 + d^6*(-28 + 48d - 21 d^2), then mask
            d3 = t2("d3"); d6 = t2("d6"); q = t2("q"); env = t2("env")
            nc.vector.tensor_tensor(d3[:], d2[:], d[:], op=AL.mult)
            nc.vector.tensor_tensor(d6[:], d3[:], d3[:], op=AL.mult)
            ts1 = t2("ts1")
            nc.vector.tensor_scalar(ts1[:], d[:], 48.0, None, op0=AL.mult)
            nc.vector.scalar_tensor_tensor(q[:], d2[:], -21.0, ts1[:],
                                           op0=AL.mult, op1=AL.add)
            nc.vector.tensor_scalar(q[:], q[:], -28.0, None, op0=AL.add)
            nc.vector.tensor_tensor(env[:], d6[:], q[:], op=AL.mult)
            nc.vector.tensor_scalar(env[:], env[:], 1.0, None, op0=AL.add)
            nc.vector.tensor_tensor(env[:], env[:], mask[:], op=AL.mult)
            # sines via recurrence: s1=sin(pi d), c=sin(pi d + pi/2)
            nc.scalar.activation(geo[:, :, 16], d[:], AF.Sin, scale=math.pi)
            c1 = t2("c1")
            d05 = t2("d05")
            nc.vector.tensor_scalar(d05[:], d[:], 0.5, None, op0=AL.add)
            nc.scalar.activation(c1[:], d05[:], AF.Sin, scale=math.pi)
            nc.vector.tensor_scalar(c1[:], c1[:], 2.0, None, op0=AL.mult)
            nc.vector.tensor_tensor(geo[:, :, 17], c1[:], geo[:, :, 16], op=AL.mult)
            for n in range(3, 9):
                sn = t2("sn")
                nc.vector.tensor_tensor(sn[:], c1[:], geo[:, :, 14+n], op=AL.mult)
                nc.vector.tensor_tensor(geo[:, :, 15+n], sn[:], geo[:, :, 13+n],
                                        op=AL.subtract)
            renv = t2("renv")
            nc.vector.tensor_tensor(renv[:], env[:], rec[:], op=AL.mult)
            nc.vector.tensor_scalar(renv[:], renv[:], math.sqrt(2.0), None, op0=AL.mult)
            nc.vector.tensor_tensor(
                geo[:, :, 16:24], geo[:, :, 16:24],
                renv[:, :, None].broadcast_to((128, NT, 8)), op=AL.mult)
            # spherical harmonics into geo cols 0..15
            ux, uy, uz = u[:, :, 0], u[:, :, 1], u[:, :, 2]
            nc.vector.memset(geo[:, :, 0], 1.0)
            nc.scalar.mul(geo[:, :, 1], uy, S3)
            nc.scalar.mul(geo[:, :, 2], uz, S3)
            nc.scalar.mul(geo[:, :, 3], ux, S3)
            xy = t2("xy"); yz = t2("yz"); xz = t2("xz")
            x2 = t2("x2"); y2 = t2("y2"); z2 = t2("z2"); xmy = t2("xmy")
            nc.vector.tensor_tensor(xy[:], ux, uy, op=AL.mult)
            nc.vector.tensor_tensor(yz[:], uy, uz, op=AL.mult)
            nc.vector.tensor_tensor(xz[:], ux, uz, op=AL.mult)
            nc.vector.tensor_tensor(x2[:], ux, ux, op=AL.mult)
            nc.vector.tensor_tensor(y2[:], uy, uy, op=AL.mult)
            nc.vector.tensor_tensor(z2[:], uz, uz, op=AL.mult)
            nc.vector.tensor_tensor(xmy[:], x2[:], y2[:], op=AL.subtract)
            nc.scalar.mul(geo[:, :, 4], xy[:], S15)
            nc.scalar.mul(geo[:, :, 5], yz[:], S15)
            nc.vector.tensor_scalar(geo[:, :, 6], z2[:], 1.5*S5, 0.5*S5,
                                    op0=AL.mult, op1=AL.subtract)
            nc.scalar.mul(geo[:, :, 7], xz[:], S15)
            nc.scalar.mul(geo[:, :, 8], xmy[:], 0.5*S15)
            g1 = t2("g1")
            nc.vector.scalar_tensor_tensor(g1[:], x2[:], 3.0, y2[:],
                                           op0=AL.mult, op1=AL.subtract)
            nc.vector.tensor_tensor(g1[:], g1[:], uy, op=AL.mult)
            nc.scalar.mul(geo[:, :, 9], g1[:], S358)
            g2 = t2("g2")
            nc.vector.tensor_tensor(g2[:], xy[:], uz, op=AL.mult)
            nc.scalar.mul(geo[:, :, 10], g2[:], S105)
            fz = t2("fz")
            nc.vector.tensor_scalar(fz[:], z2[:], 5.0, 1.0, op0=AL.mult,
                                    op1=AL.subtract)
            g3 = t2("g3")
            nc.vector.tensor_tensor(g3[:], fz[:], uy, op=AL.mult)
            nc.scalar.mul(geo[:, :, 11], g3[:], S218)
            f2 = t2("f2")
            nc.vector.tensor_scalar(f2[:], fz[:], -2.0, None, op0=AL.add)
            g4 = t2("g4")
            nc.vector.tensor_tensor(g4[:], f2[:], uz, op=AL.mult)
            nc.scalar.mul(geo[:, :, 12], g4[:], 0.5*S7)
            g5 = t2("g5")
            nc.vector.tensor_tensor(g5[:], fz[:], ux, op=AL.mult)
            nc.scalar.mul(geo[:, :, 13], g5[:], S218)
            g6 = t2("g6")
            nc.vector.tensor_tensor(g6[:], xmy[:], uz, op=AL.mult)
            nc.scalar.mul(geo[:, :, 14], g6[:], 0.5*S105)
            g7 = t2("g7")
            nc.vector.tensor_tensor(g7[:], xmy[:], ux, op=AL.mult)
            nc.scalar.mul(geo[:, :, 15], g7[:], S358)
            # env into geo col 24 for the transpose
            nc.vector.tensor_copy(geo[:, :, 24], env[:])
            # per-tile transposes -> feature-major smalls
            for t in range(NT):
                tc_ = slice(t*128, (t+1)*128)
                psy = pst.tile([16, 128], F32, tag="ps", name="psy")
                nc.tensor.transpose(psy[:], geo[:, t, 0:16], W["iden"][:])
                nc.vector.tensor_copy(ybf[:, tc_], psy[:])
                psb = pst.tile([8, 128], F32, tag="ps", name="psb")
                nc.tensor.transpose(psb[:], geo[:, t, 16:24], W["iden"][:])
                nc.vector.tensor_copy(bbf[:, tc_], psb[:])
                pse = pst.tile([1, 128], F32, tag="ps", name="pse")
                nc.tensor.transpose(pse[:], geo[:, t, 24:25], W["iden"][:])
                nc.vector.tensor_copy(envbf[:, tc_], pse[:])

            def env_chunk(c0, cn):
                """ones-broadcast env over 128 partitions -> bf16 SBUF tile."""
                pe = pst.tile([128, 512], F32, tag="ps", name="pe")
                nc.tensor.matmul(pe[:, 0:cn], W["ones1"][:], envbf[:, c0:c0+cn],
                                 start=True, stop=True)
                esb = chp.tile([128, 512], BF16, tag="b512", name="esb")
                nc.vector.tensor_copy(esb[:, 0:cn], pe[:, 0:cn])
                return esb

            # ================= two-body MLP -> xsb
            for (c0, cn) in CH:
                psA = pst.tile([32, 512], F32, tag="ps", name="psA")
                nc.tensor.matmul(psA[:, 0:cn], W["w1b"][:], bbf[:, c0:c0+cn],
                                 start=True, stop=False)
                oht = chp.tile([128, 512], BF16, tag="b512", name="oht")
                nc.sync.dma_start(oht[:, 0:cn], D["ohs"][:, c0:c0+cn])
                nc.tensor.matmul(psA[:, 0:cn], W["tabs"][:], oht[:, 0:cn],
                                 start=False, stop=False)
                oht2 = chp.tile([128, 512], BF16, tag="b512", name="oht2")
                nc.sync.dma_start(oht2[:, 0:cn], D["ohr"][:, c0:c0+cn])
                nc.tensor.matmul(psA[:, 0:cn], W["tabr"][:], oht2[:, 0:cn],
                                 start=False, stop=True)
                x1 = chp.tile([32, 512], BF16, tag="f512", name="x1")
                nc.scalar.activation(x1[:, 0:cn], psA[:, 0:cn], AF.Silu)
                psB = pst.tile([64, 512], F32, tag="ps", name="psB")
                nc.tensor.matmul(psB[:, 0:cn], W["wtb2"][:], x1[:, 0:cn],
                                 start=True, stop=True)
                x2c = chp.tile([64, 512], BF16, tag="f512", name="x2c")
                nc.scalar.activation(x2c[:, 0:cn], psB[:, 0:cn], AF.Silu)
                psC = pst.tile([128, 512], F32, tag="ps", name="psC")
                nc.tensor.matmul(psC[:, 0:cn], W["wtb3"][:], x2c[:, 0:cn],
                                 start=True, stop=True)
                x3 = chp.tile([128, 512], BF16, tag="f512", name="x3")
                nc.scalar.activation(x3[:, 0:cn], psC[:, 0:cn], AF.Silu)
                esb = env_chunk(c0, cn)
                for oc in range(2):
                    psD = pst.tile([128, 512], F32, tag="ps", name="psD")
                    nc.tensor.matmul(psD[:, 0:cn], W["wtb4"][:, oc*128:(oc+1)*128],
                                     x3[:, 0:cn], start=True, stop=True)
                    nc.vector.tensor_tensor(xsb[:, oc, c0:c0+cn], psD[:, 0:cn],
                                            esb[:, 0:cn], op=AL.mult)

            # ================= shared per-layer pieces
            def wproj_16(wname, c0, cn, psv):
                """16-dim projection of x (feature-major out at partitions 0..15)"""
                nc.tensor.matmul(psv[0:16, 0:cn], W[wname][:, 0, :], xsb[:, 0, c0:c0+cn],
                                 start=True, stop=False)
                nc.tensor.matmul(psv[0:16, 0:cn], W[wname][:, 1, :], xsb[:, 1, c0:c0+cn],
                                 start=False, stop=True)

            def scatter_layer(wname, node_dst):
                """w=x@W per tile, M=w(x)Y, node += M^T S; node_dst [128,2,256] bf16."""
                nps = [psacc.tile([128, 256], F32, tag=f"pout{oc}", name=f"nps{oc}_{wname}") for oc in range(2)]
                for t in range(NT):
                    tc_ = slice(t*128, (t+1)*128)
                    wps = pst.tile([128, 16], F32, tag="ps", name="wps")
                    nc.tensor.matmul(wps[:], xsb[:, 0, tc_], W[wname][:, 0, :],
                                     start=True, stop=False)
                    nc.tensor.matmul(wps[:], xsb[:, 1, tc_], W[wname][:, 1, :],
                                     start=False, stop=True)
                    mbf = chp.tile([128, 16, 16], BF16, tag="mbf", name="mbf")
                    nc.vector.tensor_tensor(
                        mbf[:], wps[:, :, None].broadcast_to((128, 16, 16)),
                        geo[:, t, None, 0:16].broadcast_to((128, 16, 16)), op=AL.mult)
                    stile = chp.tile([128, 256], BF16, tag="b512", name="stile")
                    nc.sync.dma_start(stile[:], D["smat"][t, :, :])
                    mview = mbf[:].rearrange("p a b -> p (a b)")
                    for oc in range(2):
                        nc.tensor.matmul(nps[oc][:], mview[:, oc*128:(oc+1)*128],
                                         stile[:], start=(t == 0), stop=(t == NT-1))
                # transpose node (c, n) -> (n, c), cast bf16
                nsb = tmp.tile([128, 2, 256], F32, tag="nsb", name="nsb")
                for oc in range(2):
                    nc.vector.tensor_copy(nsb[:, oc, :], nps[oc][:])
                for oc in range(2):
                    for nk in range(2):
                        pstr = pst.tile([128, 128], F32, tag="ps", name="pstr")
                        nc.tensor.transpose(pstr[:], nsb[:, oc, nk*128:(nk+1)*128],
                                            W["iden"][:])
                        nc.vector.tensor_copy(node_dst[:, nk, oc*128:(oc+1)*128],
                                              pstr[:])

            # ================= layer 1: scatter, gather, TP
            scatter_layer("ww1", node_nm)
            for (c0, cn) in CH:
                psv = pst.tile([16, 512], F32, tag="ps", name="psv")
                wproj_16("ww0", c0, cn, psv)
                nc.vector.tensor_copy(v16[:, c0:c0+cn], psv[0:16, 0:cn])

            for (c0, cn) in CH:
                # Yrep
                yrep = chp.tile([128, 2, 512], BF16, tag="yrep", name="yrep")
                for oc in range(2):
                    psy2 = pst.tile([128, 512], F32, tag="ps", name="psy2")
                    nc.tensor.matmul(psy2[:, 0:cn], W["repj"][:, oc*128:(oc+1)*128],
                                     ybf[:, c0:c0+cn], start=True, stop=True)
                    nc.scalar.copy(yrep[:, oc, 0:cn], psy2[:, 0:cn])
                # gather + v-fold -> wYv
                gch = chp.tile([128, 2, 512], BF16, tag="gch", name="gch")
                nc.sync.dma_start(gch[:, :, 0:cn], D["gmat"][:, :, c0:c0+cn])
                wyv = chp.tile([128, 2, 512], BF16, tag="wyv", name="wyv")
                for oc in range(2):
                    pw = pst.tile([128, 512], F32, tag="ps", name="pw")
                    for kc in range(2):
                        nc.tensor.matmul(pw[:, 0:cn], node_nm[:, kc, oc*128:(oc+1)*128],
                                         gch[:, kc, 0:cn], start=(kc == 0), stop=(kc == 1))
                    pv = pst.tile([128, 512], F32, tag="ps", name="pv")
                    nc.tensor.matmul(pv[:, 0:cn], W["e16b"][:, oc*128:(oc+1)*128],
                                     v16[:, c0:c0+cn], start=True, stop=True)
                    vsb = chp.tile([128, 512], BF16, tag="b512", name="vsb")
                    nc.scalar.copy(vsb[:, 0:cn], pv[:, 0:cn])
                    nc.vector.tensor_tensor(wyv[:, oc, 0:cn], pw[:, 0:cn],
                                            vsb[:, 0:cn], op=AL.mult)
                # per-m products and C contraction
                pouts = [psacc.tile([128, 512], F32, tag=f"pout{i}", name=f"pout{i}_{c0}") for i in range(3)]
                for m in range(MUL):
                    pbf = chp.tile([128, 2, 512], BF16, tag="pbf", name="pbf")
                    for oc in range(2):
                        pr = psr.tile([128, 512], F32, tag="pr", name="pr")
                        nc.tensor.matmul(pr[:, 0:cn],
                                         W["repibig"][:, m % 8, oc*128:(oc+1)*128],
                                         wyv[:, m // 8, 0:cn], start=True, stop=True)
                        nc.vector.tensor_tensor(pbf[:, oc, 0:cn], pr[:, 0:cn],
                                                yrep[:, oc, 0:cn], op=AL.mult)
                    for o3, (cs, cw) in enumerate(((0, 128), (128, 128), (256, 48))):
                        for kc in range(2):
                            nc.tensor.matmul(
                                pouts[o3][0:cw, 0:cn],
                                cst[:, kc, m, cs:cs+cw],
                                pbf[:, kc, 0:cn],
                                start=(m == 0 and kc == 0),
                                stop=(m == MUL-1 and kc == 1))
                nc.scalar.copy(vnpA[:, c0:c0+cn], pouts[0][:, 0:cn])
                nc.scalar.copy(vnpB[:, c0:c0+cn], pouts[1][:, 0:cn])
                nc.scalar.copy(vnpC[:, c0:c0+cn], pouts[2][0:48, 0:cn])

            # ================= MLP block (shared for both layers)
            def mlp(wl_a, wl_t, t_extra, wl_b, wl_c, tk):
                for (c0, cn) in CH:
                    h1 = hp.tile([128, 2, 512], BF16, tag="h1", name="h1")
                    for oc in range(2):
                        ph = pst.tile([128, 512], F32, tag="ps", name="ph")
                        ocs = slice(oc*128, (oc+1)*128)
                        nc.tensor.matmul(ph[:, 0:cn], W[wl_a][:, 0, ocs],
                                         xsb[:, 0, c0:c0+cn], start=True, stop=False)
                        nc.tensor.matmul(ph[:, 0:cn], W[wl_a][:, 1, ocs],
                                         xsb[:, 1, c0:c0+cn], start=False, stop=False)
                        nc.tensor.matmul(ph[:, 0:cn], W[wl_t][:, ocs],
                                         t_extra[0:tk, c0:c0+cn], start=False, stop=True)
                        nc.scalar.activation(h1[:, oc, 0:cn], ph[:, 0:cn], AF.Silu)
                    h2 = hp.tile([128, 2, 512], BF16, tag="h2", name="h2")
                    for oc in range(2):
                        ph2 = pst.tile([128, 512], F32, tag="ps", name="ph2")
                        ocs = slice(oc*128, (oc+1)*128)
                        for kc in range(2):
                            nc.tensor.matmul(ph2[:, 0:cn], W[wl_b][:, kc, ocs],
                                             h1[:, kc, 0:cn], start=(kc == 0),
                                             stop=(kc == 1))
                        nc.scalar.activation(h2[:, oc, 0:cn], ph2[:, 0:cn], AF.Silu)
                    esb = env_chunk(c0, cn)
                    for oc in range(2):
                        ph3 = pst.tile([128, 512], F32, tag="ps", name="ph3")
                        ocs = slice(oc*128, (oc+1)*128)
                        for kc in range(2):
                            nc.tensor.matmul(ph3[:, 0:cn], W[wl_c][:, kc, ocs],
                                             h2[:, kc, 0:cn], start=(kc == 0),
                                             stop=(kc == 1))
                        ysb = chp.tile([128, 512], BF16, tag="f512", name="ysb")
                        nc.vector.tensor_tensor(ysb[:, 0:cn], ph3[:, 0:cn],
                                                esb[:, 0:cn], op=AL.mult)
                        nc.vector.scalar_tensor_tensor(
                            xsb[:, oc, c0:c0+cn], xsb[:, oc, c0:c0+cn], 0.5,
                            ysb[:, 0:cn], op0=AL.mult, op1=AL.add)

            mlp("wl11", "wl11t", vnpA, "wl12", "wl13", 64)

            # ================= layer 2
            node2 = perm.tile([128, 2, 256], BF16, tag="node2", name="node2")
            scatter_layer("ww2", node2)
            for t in range(NT):
                tc_ = slice(t*128, (t+1)*128)
                gt = chp.tile([128, 2, 128], BF16, tag="b512", name="gt")
                nc.sync.dma_start(gt[:], D["gmat"][:, :, tc_])
                pw2 = pst.tile([128, 256], F32, tag="ps", name="pw2")
                for kc in range(2):
                    nc.tensor.matmul(pw2[:], gt[:, kc, :], node2[:, kc, :],
                                     start=(kc == 0), stop=(kc == 1))
                vne = chp.tile([128, 240], BF16, tag="b512", name="vne")
                ptrA = pst.tile([128, 64], BF16, tag="ps", name="ptrA")
                nc.tensor.transpose(ptrA[:], vnpA[64:128, tc_], W["idenb64"][64:128, :])
                nc.vector.tensor_copy(vne[:, 0:64], ptrA[:])
                ptrB = pst.tile([128, 128], BF16, tag="ps", name="ptrB")
                nc.tensor.transpose(ptrB[:], vnpB[:, tc_], W["idenb"][:])
                nc.vector.tensor_copy(vne[:, 64:192], ptrB[:])
                ptrC = pst.tile([128, 48], BF16, tag="ps", name="ptrC")
                nc.tensor.transpose(ptrC[:], vnpC[:, tc_], W["idenb"][0:48, 0:48])
                nc.vector.tensor_copy(vne[:, 192:240], ptrC[:])
                prod = chp.tile([128, 16, 15], F32, tag="f256", name="prod")
                wyview = pw2[:].rearrange("p (a b) -> p a b", a=16)
                nc.vector.tensor_tensor(prod[:], wyview[:, :, 1:16],
                                        vne[:].rearrange("p (a b) -> p a b", a=16),
                                        op=AL.mult)
                t02 = chp.tile([128, 48], F32, tag="f256", name="t02")
                for li, l in enumerate((1, 2, 3)):
                    lo = SLSTART[l] - 1
                    nc.vector.tensor_reduce(
                        t02[:, li*16:(li+1)*16],
                        prod[:, :, lo:lo+2*l+1],
                        axis=mybir.AxisListType.X, op=AL.add)
                ptt = pst.tile([48, 128], F32, tag="ps", name="ptt")
                nc.tensor.transpose(ptt[:], t02[:], W["iden"][:])
                nc.vector.tensor_copy(tp02[:, tc_], ptt[:])

            mlp("wl21", "wl21t", tp02, "wl22", "wl23", 48)

            # ================= head
            for (c0, cn) in CH:
                psh = pst.tile([128, 512], F32, tag="ps", name="psh")
                for kc in range(2):
                    nc.tensor.matmul(psh[:, 0:cn], W["wh"][:, kc, :],
                                     xsb[:, kc, c0:c0+cn], start=(kc == 0),
                                     stop=(kc == 1))
                esb = env_chunk(c0, cn)
                xh = chp.tile([128, 512], BF16, tag="f512", name="xh")
                nc.vector.tensor_tensor(xh[:, 0:cn], psh[:, 0:cn], esb[:, 0:cn],
                                        op=AL.mult)
                pso = pst.tile([1, 512], F32, tag="ps", name="pso")
                nc.tensor.matmul(pso[:, 0:cn], W["wout"][:], xh[:, 0:cn],
                                 start=True, stop=True)
                osb = chp.tile([1, 512], F32, tag="f512", name="osb")
                nc.vector.tensor_copy(osb[:, 0:cn], pso[:, 0:cn])
                nc.sync.dma_start(D["outv"][:, c0:c0+cn], osb[:, 0:cn])

    nc.compile()
    return nc


# ---------------------------------------------------------------- host side


def _to_em(a, NT):
    """[CAP, ...] -> [128, NT, ...] edge-major (edge = t*128+p -> row p col t)."""
    return np.ascontiguousarray(a.reshape(NT, 128, *a.shape[1:]).swapaxes(0, 1))


def _prep_inputs(inputs):
    inputs = {k: np.asarray(v) for k, v in inputs.items()}
    senders = inputs["senders"].astype(np.int64)
    receivers = inputs["receivers"].astype(np.int64)
    species = inputs["species"].astype(np.int64)
    vectors = inputs["vectors"].astype(np.float32)
    eps = 1.0 / math.sqrt(1.0 + float(inputs["varepsilon"])**2)
    a2 = float(inputs["alpha"])**2

    core_of = senders // NPC
    idxs = [np.nonzero(core_of == c)[0] for c in range(N_CORES)]
    maxk = max(len(i) for i in idxs)
    CAP = ((maxk + 127) // 128) * 128
    NT = CAP // 128

    sc = 1.0 / math.sqrt(N_RBF + 2*EMB)
    emb = inputs["emb"].astype(np.float64)
    tabS = np.zeros((128, 32), np.float64)
    tabR = np.zeros((128, 32), np.float64)
    tabS[:100] = emb @ (inputs["W_tb1"][N_RBF:N_RBF+EMB].astype(np.float64) * sc)
    tabR[:100] = emb @ (inputs["W_tb1"][N_RBF+EMB:].astype(np.float64) * sc)

    # shared weights (scaled)
    shared = {
        "w1b": (inputs["W_tb1"][:N_RBF] * sc).astype(BF),
        "tabs": tabS.astype(BF), "tabr": tabR.astype(BF),
        "wtb2": (inputs["W_tb2"] / math.sqrt(32)).astype(BF),
        "wtb3": (inputs["W_tb3"] / math.sqrt(64)).astype(BF),
        "wtb4": (inputs["W_tb4"] / math.sqrt(128)).astype(BF),
        "ww0": ((inputs["W_w0"] / math.sqrt(HIDDEN)).reshape(2, 128, 16).swapaxes(0, 1)).astype(BF),
        "ww1": ((inputs["W_w1"] * eps / math.sqrt(HIDDEN)).reshape(2, 128, 16).swapaxes(0, 1)).astype(BF),
        "ww2": ((inputs["W_w2"] * eps / math.sqrt(HIDDEN)).reshape(2, 128, 16).swapaxes(0, 1)).astype(BF),
        "wl12": ((inputs["W_l12"] / math.sqrt(HIDDEN)).reshape(2, 128, 256).swapaxes(0, 1)).astype(BF),
        "wl13": ((inputs["W_l13"] / math.sqrt(HIDDEN) * a2 / (1 + a2)).reshape(2, 128, 256).swapaxes(0, 1)).astype(BF),
        "wl22": ((inputs["W_l22"] / math.sqrt(HIDDEN)).reshape(2, 128, 256).swapaxes(0, 1)).astype(BF),
        "wl23": ((inputs["W_l23"] / math.sqrt(HIDDEN) * a2 / (1 + a2)).reshape(2, 128, 256).swapaxes(0, 1)).astype(BF),
        "wh": ((inputs["W_h"] / math.sqrt(HIDDEN)).reshape(2, 128, 128).swapaxes(0, 1)).astype(BF),
        "wout": (inputs["W_out"] / math.sqrt(128)).astype(BF),
    }
    s320 = 1.0 / math.sqrt(320)
    wl11 = inputs["W_l11"] * s320
    shared["wl11"] = wl11[:256].reshape(2, 128, 256).swapaxes(0, 1).astype(BF)
    shared["wl11t"] = wl11[256:320].astype(BF)
    wl21 = inputs["W_l21"] * s320
    shared["wl21"] = wl21[:256].reshape(2, 128, 256).swapaxes(0, 1).astype(BF)
    shared["wl21t"] = wl21[256+16:320].astype(BF)     # drop zero p0 block

    shared["ttT"] = np.zeros((128, 2, 256), np.float32)
    tt = _build_ttT()
    shared["ttT"][:, 0, :] = tt[0:128]
    shared["ttT"][0:32, 1, :] = tt[128:160]
    shared["ttT"] = shared["ttT"].astype(BF)
    shared["wk"] = _build_wk(inputs["W_v1"], inputs["W_v2"], inputs["W_v3"]).astype(BF)

    repj = np.zeros((16, 256), np.float32)
    for i in range(16):
        for j in range(16):
            repj[j, i*16+j] = 1.0
    repibig = np.zeros((128, 8, 256), np.float32)
    for g in range(8):
        for i in range(16):
            for j in range(16):
                repibig[g*16+i, g, i*16+j] = 1.0
    e16b = np.zeros((16, 256), np.float32)
    for m in range(16):
        e16b[m, m*16:(m+1)*16] = 1.0
    shared["repj"] = repj.astype(BF)
    shared["repibig"] = repibig.astype(BF)
    shared["e16b"] = e16b.astype(BF)
    shared["iden"] = np.eye(128, dtype=np.float32)
    shared["idenb"] = np.eye(128, dtype=np.float32).astype(BF)
    i64 = np.zeros((128, 64), np.float32)
    i64[64 + np.arange(64), np.arange(64)] = 1.0
    shared["idenb64"] = i64.astype(BF)
    shared["ones1"] = np.ones((1, 128), np.float32).astype(BF)

    for k in list(shared):
        if shared[k].dtype not in (np.dtype(np.float32), np.dtype(BF)):
            shared[k] = shared[k].astype(np.float32)
        shared[k] = np.ascontiguousarray(shared[k])

    in_maps = []
    for c in range(N_CORES):
        idx = idxs[c]
        k = len(idx)
        vec = np.zeros((CAP, 3), np.float32); vec[:, 2] = 0.5
        vec[:k] = vectors[idx]
        maskv = np.zeros(CAP, np.float32); maskv[:k] = 1.0
        sl = np.zeros(CAP, np.int64)
        sl[:k] = senders[idx] - c * NPC
        spe_s = np.full(CAP, 127, np.int64); spe_s[:k] = species[senders[idx]]
        spe_r = np.full(CAP, 127, np.int64); spe_r[:k] = species[receivers[idx]]
        ohs = np.zeros((128, CAP), np.float32)
        ohs[spe_s, np.arange(CAP)] = 1.0
        ohr = np.zeros((128, CAP), np.float32)
        ohr[spe_r, np.arange(CAP)] = 1.0
        smat = np.zeros((CAP, 256), np.float32)
        smat[np.arange(k), sl[:k]] = 1.0
        gmat = np.zeros((2, 128, CAP), np.float32)
        gmat[sl[:k] // 128, sl[:k] % 128, np.arange(k)] = 1.0
        m = dict(shared)
        m["vec"] = _to_em(vec, CAP // 128)
        m["maskt"] = _to_em(maskv, CAP // 128)
        m["ohs"] = ohs.astype(BF)
        m["ohr"] = ohr.astype(BF)
        m["smat"] = np.ascontiguousarray(
            smat.reshape(CAP // 128, 128, 256)).astype(BF)
        m["gmat"] = np.ascontiguousarray(gmat.swapaxes(0, 1)).astype(BF)
        m = {k2: np.ascontiguousarray(v) for k2, v in m.items()}
        in_maps.append(m)
    return in_maps, idxs, CAP


def _run(inputs, trace=False, tmpdir=None):
    in_maps, idxs, CAP = _prep_inputs(inputs)
    if CAP not in _PROG_CACHE:
        _PROG_CACHE[CAP] = _build_program(CAP)
    nc = _PROG_CACHE[CAP]
    res = run_bass_kernel_spmd(nc, in_maps, list(range(N_CORES)), trace=trace,
                               tmpdir=tmpdir)
    out = np.zeros((E, 1), np.float32)
    for c in range(N_CORES):
        k = len(idxs[c])
        out[idxs[c], 0] = res.results[c]["outv"][0, :k]
    return out, res


def kernel(**inputs):
    out, _ = _run(inputs, trace=False)
    return out



# revision 8
# speedup vs baseline: 1.6724x; 1.6724x over previous
"""Allegro GNN on 8 TRN2 NeuronCores — Bass/Tile kernel (j-major TP rewrite).

Sharding: nodes are partitioned across cores (256 nodes/core); every edge is
routed to the core owning its *sender* node, so the scatter-sum over senders
and the gather-back are both core-local: zero collectives.

Host precomputes per-edge geometry (Y, bessel*env, env) and the two-body
first-layer pre-activation (table gathers), plus one-hot scatter/gather
matrices and the folded Clebsch-Gordan contraction matrices Cj.

Device program (feature-major activations):
  pass1: two-body MLP -> xsb; v16; scatter1 (node-major accumulate)
  pass2 per chunk: gather -> wyv; tp0 (diagonal split); j-major TP
         Vn' = sum_j Cj^T (wyv * Y_j); mlp1
  pass3: scatter2 (240-col node-major)
  pass4 per chunk: gather2 -> tp0_2 via R2; mlp2; head
"""
import math
import sys

import numpy as np

sys.path.insert(0, "/opt/trn_rl_repo")

import concourse.bacc as bacc  # noqa: E402
import concourse.mybir as mybir  # noqa: E402
from concourse import tile  # noqa: E402
from concourse.bass_utils import run_bass_kernel_spmd  # noqa: E402
import ml_dtypes  # noqa: E402

F32 = mybir.dt.float32
BF16 = mybir.dt.bfloat16
BF = ml_dtypes.bfloat16
AL = mybir.AluOpType
AF = mybir.ActivationFunctionType

E, NNODE = 32768, 2048
NUM_SPECIES, EMB = 100, 32
MUL, HIDDEN, N_RBF, LMAX = 16, 256, 8, 3
N_CORES = 8
NPC = NNODE // N_CORES          # nodes per core
SLSTART = {0: 0, 1: 1, 2: 4, 3: 9}

# ---------------------------------------------------------------- CG tensors


def _cg(j1, m1, j2, m2, j3, m3):
    if m1 + m2 != m3:
        return 0.0
    f = math.factorial
    pre = math.sqrt((2*j3+1) * f(j1+j2-j3) * f(j1-j2+j3) * f(-j1+j2+j3) / f(j1+j2+j3+1))
    pre *= math.sqrt(f(j3+m3)*f(j3-m3)*f(j1-m1)*f(j1+m1)*f(j2-m2)*f(j2+m2))
    s = 0.0
    kmin = max(0, j2 - j3 - m1, j1 - j3 + m2)
    kmax = min(j1 + j2 - j3, j1 - m1, j2 + m2)
    for k in range(kmin, kmax + 1):
        s += (-1)**k / (f(k)*f(j1+j2-j3-k)*f(j1-m1-k)*f(j2+m2-k)*f(j3-j2+m1+k)*f(j3-j1-m2+k))
    return pre * s


def _umat(l):
    U = np.zeros((2*l+1, 2*l+1), dtype=complex)
    U[l, l] = 1.0
    s2 = 1.0 / math.sqrt(2.0)
    for m in range(1, l + 1):
        U[l+m, l-m] = s2
        U[l+m, l+m] = (-1)**m * s2
        U[l-m, l-m] = 1j * s2
        U[l-m, l+m] = -1j * (-1)**m * s2
    return U


def _real_coupling(l1, l2, l3):
    C = np.zeros((2*l1+1, 2*l2+1, 2*l3+1), dtype=complex)
    for a, m1 in enumerate(range(-l1, l1+1)):
        for b, m2 in enumerate(range(-l2, l2+1)):
            for c, m3 in enumerate(range(-l3, l3+1)):
                C[a, b, c] = _cg(l1, m1, l2, m2, l3, m3)
    T = np.einsum('am,bn,ck,mnk->abc', _umat(l1), _umat(l2), _umat(l3).conj(), C)
    Tr, Ti = np.real(T), np.imag(T)
    T = Tr if np.linalg.norm(Tr) >= np.linalg.norm(Ti) else Ti
    n = np.linalg.norm(T)
    return None if n < 1e-8 else (T / n).astype(np.float32)


PATHS = {l3: [] for l3 in range(LMAX + 1)}
for _l1 in range(LMAX + 1):
    for _l2 in range(LMAX + 1):
        for _l3 in range(abs(_l1 - _l2), min(_l1 + _l2, LMAX) + 1):
            _T = _real_coupling(_l1, _l2, _l3)
            if _T is not None:
                PATHS[_l3].append((_l1, _l2, _T))
NPATH = {l3: len(PATHS[l3]) for l3 in range(LMAX + 1)}


def _build_cj(W_v1, W_v2, W_v3):
    """Cj[j, (m,i)=256, (d,k-1)=240] with T2-fold, scales, Wv baked in."""
    T2 = {l1: T[:, :, 0] for (l1, l2, T) in PATHS[0]}
    Wv = {1: W_v1.astype(np.float64), 2: W_v2.astype(np.float64),
          3: W_v3.astype(np.float64)}
    C = np.zeros((16, 256, 240), np.float64)
    for l3 in (1, 2, 3):
        scale = math.sqrt(2 * l3 + 1) / math.sqrt(MUL * NPATH[l3])
        for p, (l1, l2, T) in enumerate(PATHS[l3]):
            i0, j0 = SLSTART[l1], SLSTART[l2]
            Tf = np.einsum('kq,ijq->ijk', T2[l3].astype(np.float64),
                           T.astype(np.float64))
            # cols: (d, kabs-1); rows: (m, i); j index: j0+jj
            for k in range(2 * l3 + 1):
                kabs = SLSTART[l3] + k
                for ii in range(2 * l1 + 1):
                    for jj in range(2 * l2 + 1):
                        t = Tf[ii, jj, k] * scale
                        if t == 0.0:
                            continue
                        # C[j0+jj, m*16+i0+ii, d*15+kabs-1] += Wv[p*16+m, d]*t
                        rows = np.arange(MUL) * 16 + (i0 + ii)
                        C[j0 + jj, rows[:, None], np.arange(MUL) * 15 + (kabs - 1)] \
                            += Wv[l3][np.arange(MUL)[:, None] + p * 16,
                                      np.arange(MUL)] * t
    return C


def _build_r1():
    """R1[(m,i)=256, (p,m)=64]: tp0_1 = R1^T @ (wyv * Yrep_i)."""
    R = np.zeros((256, 64), np.float64)
    for p, (l1, l2, T) in enumerate(PATHS[0]):
        c = T[0, 0, 0] if l1 == 0 else T[1, 1, 0]
        i0 = SLSTART[l1]
        for m in range(MUL):
            for ii in range(2 * l1 + 1):
                R[m * 16 + i0 + ii, p * 16 + m] = c
    return R


def _build_r2():
    """R2[(m,k-1)=240, (l,m)=48]: tp0_2 = R2^T @ (wY2' * Vn')."""
    R = np.zeros((240, 48), np.float64)
    for li, l in enumerate((1, 2, 3)):
        for m in range(MUL):
            for k in range(2 * l + 1):
                kabs = SLSTART[l] + k
                R[m * 15 + (kabs - 1), li * 16 + m] = 1.0
    return R


# ------------------------------------------------------------- device program

_PROG_CACHE = {}


def _build_program(CAP):
    NT = CAP // 128
    CH = [(s, min(512, CAP - s)) for s in range(0, CAP, 512)]
    nc = bacc.Bacc("TRN2", target_bir_lowering=False, debug=False,
                   num_devices=N_CORES)
    D = {}

    def dp(name, shape, dt=BF16, out=False):
        D[name] = nc.declare_dram_parameter(name, list(shape), dt, isOutput=out)
        return D[name]

    dp("x1pre", [32, CAP]); dp("ybfd", [16, CAP]); dp("yemd", [128, NT, 16])
    dp("envd", [1, CAP])
    dp("smat", [NT, 128, 256]); dp("gmat", [128, 2, CAP])
    dp("cjd", [128, 16, 2, 240])
    dp("r1d", [128, 2, 64]); dp("r2d", [128, 2, 48])
    dp("repmi", [16, 256]); dp("e16b", [16, 256]); dp("ones1", [1, 128])
    dp("selj", [16, 2048])
    dp("wtb2", [32, 64]); dp("wtb3", [64, 128]); dp("wtb4", [128, 256])
    dp("ww0", [128, 2, 16]); dp("ww1", [128, 2, 16]); dp("ww2", [128, 2, 16])
    dp("wl11", [128, 2, 256]); dp("wl11t", [64, 256])
    dp("wl12", [128, 2, 256]); dp("wl13", [128, 2, 256])
    dp("wl21", [128, 2, 256]); dp("wl21t", [48, 256])
    dp("wl22", [128, 2, 256]); dp("wl23", [128, 2, 256])
    dp("wh", [128, 2, 128]); dp("wout", [128, 1])
    dp("outv", [1, CAP], dt=F32, out=True)

    with tile.TileContext(nc) as tc:
        with tc.tile_pool(name="perm", bufs=1) as perm, \
             tc.tile_pool(name="wpool", bufs=1) as wpool, \
             tc.tile_pool(name="chp", bufs=2) as chp, \
             tc.tile_pool(name="chp3", bufs=3) as chp3, \
             tc.tile_pool(name="pst", bufs=4, space="PSUM") as pst, \
             tc.tile_pool(name="pacc", bufs=1, space="PSUM") as pacc, \
             tc.tile_pool(name="psr", bufs=2, space="PSUM") as psr:

            # ---- persistent SBUF
            xsb = perm.tile([128, 2, CAP], BF16, tag="xsb", name="xsb")
            envb = perm.tile([128, CAP], BF16, tag="envb", name="envb")
            ybf = perm.tile([16, CAP], BF16, tag="ybf", name="ybf")
            yem = perm.tile([128, NT, 16], BF16, tag="yem", name="yem")
            x1s = perm.tile([32, CAP], BF16, tag="x1s", name="x1s")
            envs = perm.tile([1, CAP], BF16, tag="envs", name="envs")
            v16 = perm.tile([16, CAP], BF16, tag="v16", name="v16")
            vnpA = perm.tile([128, CAP], BF16, tag="vnpA", name="vnpA")
            vnpB = perm.tile([112, CAP], BF16, tag="vnpB", name="vnpB")
            tp0sb = perm.tile([64, CAP], BF16, tag="tp0sb", name="tp0sb")
            tp02sb = perm.tile([48, CAP], BF16, tag="tp02sb", name="tp02sb")
            node1 = perm.tile([128, 2, 256], BF16, tag="node1", name="node1")
            node2 = perm.tile([128, 2, 240], BF16, tag="node2", name="node2")
            gch = perm.tile([128, 2, CAP], BF16, tag="gch", name="gch")

            nc.sync.dma_start(ybf[:], D["ybfd"][:])
            nc.sync.dma_start(yem[:], D["yemd"][:])
            nc.sync.dma_start(x1s[:], D["x1pre"][:])
            nc.sync.dma_start(envs[:], D["envd"][:])
            nc.sync.dma_start(gch[:], D["gmat"][:])

            # ---- weights in SBUF
            W = {}
            for nm, shape in [
                ("cjd", [128, 16, 2, 240]),
                ("r1d", [128, 2, 64]), ("r2d", [128, 2, 48]),
                ("repmi", [16, 256]), ("e16b", [16, 256]), ("ones1", [1, 128]),
                ("selj", [16, 2048]),
                ("wtb2", [32, 64]), ("wtb3", [64, 128]), ("wtb4", [128, 256]),
                ("ww0", [128, 2, 16]), ("ww1", [128, 2, 16]),
                ("ww2", [128, 2, 16]),
                ("wl11", [128, 2, 256]), ("wl11t", [64, 256]),
                ("wl12", [128, 2, 256]), ("wl13", [128, 2, 256]),
                ("wl21", [128, 2, 256]), ("wl21t", [48, 256]),
                ("wl22", [128, 2, 256]), ("wl23", [128, 2, 256]),
                ("wh", [128, 2, 128]), ("wout", [128, 1]),
            ]:
                W[nm] = wpool.tile(shape, BF16, tag="w_" + nm, name="w_" + nm)
                nc.sync.dma_start(W[nm][:], D[nm][:])

            # ---- env broadcast to 128 partitions
            for (c0, cn) in CH:
                pe = pst.tile([128, 512], F32, tag="ps", name="pe")
                nc.tensor.matmul(pe[:, 0:cn], W["ones1"][:], envs[:, c0:c0+cn],
                                 start=True, stop=True)
                nc.scalar.copy(envb[:, c0:c0+cn], pe[:, 0:cn])

            # ================= pass 1: two-body MLP, v16, scatter1
            nps1 = [pacc.tile([128, 256], F32, tag=f"acc{h}", name=f"nps1_{h}")
                    for h in range(2)]
            for ci, (c0, cn) in enumerate(CH):
                x1 = chp.tile([32, 512], BF16, tag="x1", name="x1")
                nc.scalar.activation(x1[:, 0:cn], x1s[:, c0:c0+cn], AF.Silu)
                psB = pst.tile([64, 512], F32, tag="ps", name="psB")
                nc.tensor.matmul(psB[:, 0:cn], W["wtb2"][:], x1[:, 0:cn],
                                 start=True, stop=True)
                x2 = chp.tile([64, 512], BF16, tag="x2", name="x2")
                nc.scalar.activation(x2[:, 0:cn], psB[:, 0:cn], AF.Silu)
                psC = pst.tile([128, 512], F32, tag="ps", name="psC")
                nc.tensor.matmul(psC[:, 0:cn], W["wtb3"][:], x2[:, 0:cn],
                                 start=True, stop=True)
                x3 = chp.tile([128, 512], BF16, tag="x3", name="x3")
                nc.scalar.activation(x3[:, 0:cn], psC[:, 0:cn], AF.Silu)
                for oc in range(2):
                    psD = pst.tile([128, 512], F32, tag="ps", name="psD")
                    nc.tensor.matmul(psD[:, 0:cn], W["wtb4"][:, oc*128:(oc+1)*128],
                                     x3[:, 0:cn], start=True, stop=True)
                    nc.vector.tensor_tensor(xsb[:, oc, c0:c0+cn], psD[:, 0:cn],
                                            envb[:, c0:c0+cn], op=AL.mult)
                # v16
                psv = pst.tile([16, 512], F32, tag="ps", name="psv")
                nc.tensor.matmul(psv[:, 0:cn], W["ww0"][:, 0, :], xsb[:, 0, c0:c0+cn],
                                 start=True, stop=False)
                nc.tensor.matmul(psv[:, 0:cn], W["ww0"][:, 1, :], xsb[:, 1, c0:c0+cn],
                                 start=False, stop=True)
                nc.scalar.copy(v16[:, c0:c0+cn], psv[:, 0:cn])
                # scatter1 tiles of this chunk
                for t in range(c0 // 128, (c0 + cn) // 128):
                    tc_ = slice(t*128, (t+1)*128)
                    wps = pst.tile([128, 16], F32, tag="ps", name="wps")
                    nc.tensor.matmul(wps[:], xsb[:, 0, tc_], W["ww1"][:, 0, :],
                                     start=True, stop=False)
                    nc.tensor.matmul(wps[:], xsb[:, 1, tc_], W["ww1"][:, 1, :],
                                     start=False, stop=True)
                    mbf = chp3.tile([128, 16, 16], BF16, tag="mbf", name="mbf")
                    nc.vector.tensor_tensor(
                        mbf[:], wps[:, :, None].broadcast_to((128, 16, 16)),
                        yem[:, t, None, 0:16].broadcast_to((128, 16, 16)),
                        op=AL.mult)
                    stile = chp3.tile([128, 256], BF16, tag="stile", name="stile")
                    nc.sync.dma_start(stile[:], D["smat"][t, :, :])
                    mview = mbf[:].rearrange("p a b -> p (a b)")
                    for nh in range(2):
                        nc.tensor.matmul(nps1[nh][:], stile[:, nh*128:(nh+1)*128],
                                         mview[:], start=(t == 0), stop=(t == NT-1))
            for nh in range(2):
                nc.scalar.copy(node1[:, nh, :], nps1[nh][:])

            # ================= pass 2: gather, tp0, j-major TP, mlp1
            def mlp_chunk(c0, cn, wl_a, wl_t, t_sb, tk, wl_b, wl_c):
                h1 = chp.tile([128, 2, 512], BF16, tag="h1", name="h1")
                for oc in range(2):
                    ph = pst.tile([128, 512], F32, tag="ps", name="ph")
                    ocs = slice(oc*128, (oc+1)*128)
                    nc.tensor.matmul(ph[:, 0:cn], W[wl_a][:, 0, ocs],
                                     xsb[:, 0, c0:c0+cn], start=True, stop=False)
                    nc.tensor.matmul(ph[:, 0:cn], W[wl_a][:, 1, ocs],
                                     xsb[:, 1, c0:c0+cn], start=False, stop=False)
                    nc.tensor.matmul(ph[:, 0:cn], W[wl_t][:, ocs],
                                     t_sb[0:tk, c0:c0+cn], start=False, stop=True)
                    nc.scalar.activation(h1[:, oc, 0:cn], ph[:, 0:cn], AF.Silu)
                h2 = chp.tile([128, 2, 512], BF16, tag="h2", name="h2")
                for oc in range(2):
                    ph2 = pst.tile([128, 512], F32, tag="ps", name="ph2")
                    ocs = slice(oc*128, (oc+1)*128)
                    for kc in range(2):
                        nc.tensor.matmul(ph2[:, 0:cn], W[wl_b][:, kc, ocs],
                                         h1[:, kc, 0:cn], start=(kc == 0),
                                         stop=(kc == 1))
                    nc.scalar.activation(h2[:, oc, 0:cn], ph2[:, 0:cn], AF.Silu)
                for oc in range(2):
                    ph3 = pst.tile([128, 512], F32, tag="ps", name="ph3")
                    ocs = slice(oc*128, (oc+1)*128)
                    for kc in range(2):
                        nc.tensor.matmul(ph3[:, 0:cn], W[wl_c][:, kc, ocs],
                                         h2[:, kc, 0:cn], start=(kc == 0),
                                         stop=(kc == 1))
                    ysb = chp.tile([128, 512], BF16, tag="ysb", name="ysb")
                    nc.vector.tensor_tensor(ysb[:, 0:cn], ph3[:, 0:cn],
                                            envb[:, c0:c0+cn], op=AL.mult)
                    nc.vector.scalar_tensor_tensor(
                        xsb[:, oc, c0:c0+cn], xsb[:, oc, c0:c0+cn], 0.5,
                        ysb[:, 0:cn], op0=AL.mult, op1=AL.add)

            for (c0, cn) in CH:
                # gather wY (feature-major (m,i)), stage to SBUF, fold v
                pwsb = chp.tile([128, 2, 512], BF16, tag="pwsb", name="pwsb")
                for oc in range(2):
                    pw = pst.tile([128, 512], F32, tag="ps", name="pw")
                    for kc in range(2):
                        nc.tensor.matmul(pw[:, 0:cn],
                                         node1[:, kc, oc*128:(oc+1)*128],
                                         gch[:, kc, c0:c0+cn],
                                         start=(kc == 0), stop=(kc == 1))
                    nc.scalar.copy(pwsb[:, oc, 0:cn], pw[:, 0:cn])
                vsb = chp.tile([128, 2, 512], BF16, tag="vsb", name="vsb")
                for oc in range(2):
                    pv = pst.tile([128, 512], F32, tag="ps", name="pv")
                    nc.tensor.matmul(pv[:, 0:cn], W["e16b"][:, oc*128:(oc+1)*128],
                                     v16[:, c0:c0+cn], start=True, stop=True)
                    nc.scalar.copy(vsb[:, oc, 0:cn], pv[:, 0:cn])
                wyv = chp.tile([128, 2, 512], BF16, tag="wyv", name="wyv")
                for oc in range(2):
                    nc.vector.tensor_tensor(wyv[:, oc, 0:cn], pwsb[:, oc, 0:cn],
                                            vsb[:, oc, 0:cn], op=AL.mult)
                # tp0 path
                yrisb = chp.tile([128, 2, 512], BF16, tag="yrisb", name="yrisb")
                for oc in range(2):
                    pyi = pst.tile([128, 512], F32, tag="ps", name="pyi")
                    nc.tensor.matmul(pyi[:, 0:cn], W["repmi"][:, oc*128:(oc+1)*128],
                                     ybf[:, c0:c0+cn], start=True, stop=True)
                    nc.scalar.copy(yrisb[:, oc, 0:cn], pyi[:, 0:cn])
                tp0p = chp.tile([128, 2, 512], BF16, tag="tp0p", name="tp0p")
                for oc in range(2):
                    nc.vector.tensor_tensor(tp0p[:, oc, 0:cn], wyv[:, oc, 0:cn],
                                            yrisb[:, oc, 0:cn], op=AL.mult)
                ptp0 = pst.tile([64, 512], F32, tag="ps", name="ptp0")
                for kc in range(2):
                    nc.tensor.matmul(ptp0[:, 0:cn], W["r1d"][:, kc, :],
                                     tp0p[:, kc, 0:cn], start=(kc == 0),
                                     stop=(kc == 1))
                nc.scalar.copy(tp0sb[:, c0:c0+cn], ptp0[:, 0:cn])
                # j-major TP
                poutA = pacc.tile([128, 512], F32, tag="acc0", name="poutA")
                poutB = pacc.tile([112, 512], F32, tag="acc1", name="poutB")
                for j in range(16):
                    pyb = psr.tile([128, 512], F32, tag="pyb", name="pyb")
                    nc.tensor.matmul(pyb[:, 0:cn], W["selj"][:, j*128:(j+1)*128],
                                     ybf[:, c0:c0+cn], start=True, stop=True)
                    ybsb = chp3.tile([128, 512], BF16, tag="ybsb", name="ybsb")
                    nc.scalar.copy(ybsb[:, 0:cn], pyb[:, 0:cn])
                    ajs = chp3.tile([128, 2, 512], BF16, tag="ajs", name="ajs")
                    for kc in range(2):
                        nc.vector.tensor_tensor(ajs[:, kc, 0:cn], wyv[:, kc, 0:cn],
                                                ybsb[:, 0:cn], op=AL.mult)
                    for kc in range(2):
                        nc.tensor.matmul(poutA[:, 0:cn],
                                         W["cjd"][:, j, kc, 0:128],
                                         ajs[:, kc, 0:cn],
                                         start=(j == 0 and kc == 0),
                                         stop=(j == 15 and kc == 1))
                    for kc in range(2):
                        nc.tensor.matmul(poutB[:, 0:cn],
                                         W["cjd"][:, j, kc, 128:240],
                                         ajs[:, kc, 0:cn],
                                         start=(j == 0 and kc == 0),
                                         stop=(j == 15 and kc == 1))
                nc.scalar.copy(vnpA[:, c0:c0+cn], poutA[:, 0:cn])
                nc.scalar.copy(vnpB[:, c0:c0+cn], poutB[:, 0:cn])
                mlp_chunk(c0, cn, "wl11", "wl11t", tp0sb, 64, "wl12", "wl13")

            # ================= pass 3: scatter2
            nps2 = [pacc.tile([128, 240], F32, tag=f"acc{h}", name=f"nps2_{h}")
                    for h in range(2)]
            for t in range(NT):
                tc_ = slice(t*128, (t+1)*128)
                wps2 = pst.tile([128, 16], F32, tag="ps", name="wps2")
                nc.tensor.matmul(wps2[:], xsb[:, 0, tc_], W["ww2"][:, 0, :],
                                 start=True, stop=False)
                nc.tensor.matmul(wps2[:], xsb[:, 1, tc_], W["ww2"][:, 1, :],
                                 start=False, stop=True)
                mbf2 = chp3.tile([128, 16, 15], BF16, tag="mbf", name="mbf2")
                nc.vector.tensor_tensor(
                    mbf2[:], wps2[:, :, None].broadcast_to((128, 16, 15)),
                    yem[:, t, None, 1:16].broadcast_to((128, 16, 15)),
                    op=AL.mult)
                stile2 = chp3.tile([128, 256], BF16, tag="stile", name="stile2")
                nc.sync.dma_start(stile2[:], D["smat"][t, :, :])
                mview2 = mbf2[:].rearrange("p a b -> p (a b)")
                for nh in range(2):
                    nc.tensor.matmul(nps2[nh][:], stile2[:, nh*128:(nh+1)*128],
                                     mview2[:], start=(t == 0), stop=(t == NT-1))
            for nh in range(2):
                nc.scalar.copy(node2[:, nh, :], nps2[nh][:])

            # ================= pass 4: gather2, tp02, mlp2, head
            for (c0, cn) in CH:
                pw2a = pst.tile([128, 512], F32, tag="ps", name="pw2a")
                pw2b = pst.tile([112, 512], F32, tag="ps", name="pw2b")
                for kc in range(2):
                    nc.tensor.matmul(pw2a[:, 0:cn], node2[:, kc, 0:128],
                                     gch[:, kc, c0:c0+cn],
                                     start=(kc == 0), stop=(kc == 1))
                for kc in range(2):
                    nc.tensor.matmul(pw2b[:, 0:cn], node2[:, kc, 128:240],
                                     gch[:, kc, c0:c0+cn],
                                     start=(kc == 0), stop=(kc == 1))
                prodsb = chp.tile([128, 2, 512], BF16, tag="prodsb", name="prodsb")
                nc.vector.tensor_tensor(prodsb[:, 0, 0:cn], pw2a[:, 0:cn],
                                        vnpA[:, c0:c0+cn], op=AL.mult)
                nc.vector.tensor_tensor(prodsb[0:112, 1, 0:cn], pw2b[:, 0:cn],
                                        vnpB[:, c0:c0+cn], op=AL.mult)
                ptp02 = pst.tile([48, 512], F32, tag="ps", name="ptp02")
                nc.tensor.matmul(ptp02[:, 0:cn], W["r2d"][:, 0, :],
                                 prodsb[:, 0, 0:cn], start=True, stop=False)
                nc.tensor.matmul(ptp02[:, 0:cn], W["r2d"][0:112, 1, :],
                                 prodsb[0:112, 1, 0:cn], start=False, stop=True)
                nc.scalar.copy(tp02sb[:, c0:c0+cn], ptp02[:, 0:cn])
                mlp_chunk(c0, cn, "wl21", "wl21t", tp02sb, 48, "wl22", "wl23")
                # head
                psh = pst.tile([128, 512], F32, tag="ps", name="psh")
                for kc in range(2):
                    nc.tensor.matmul(psh[:, 0:cn], W["wh"][:, kc, :],
                                     xsb[:, kc, c0:c0+cn], start=(kc == 0),
                                     stop=(kc == 1))
                xh = chp.tile([128, 512], BF16, tag="xh", name="xh")
                nc.vector.tensor_tensor(xh[:, 0:cn], psh[:, 0:cn],
                                        envb[:, c0:c0+cn], op=AL.mult)
                pso = pst.tile([1, 512], F32, tag="ps", name="pso")
                nc.tensor.matmul(pso[:, 0:cn], W["wout"][:], xh[:, 0:cn],
                                 start=True, stop=True)
                osb = chp.tile([1, 512], F32, tag="osb", name="osb")
                nc.vector.tensor_copy(osb[:, 0:cn], pso[:, 0:cn])
                nc.sync.dma_start(D["outv"][:, c0:c0+cn], osb[:, 0:cn])

    nc.compile()
    return nc


# ---------------------------------------------------------------- host side


def _sph_harm16(u):
    x, y, z = u[:, 0], u[:, 1], u[:, 2]
    x2, y2, z2 = x * x, y * y, z * z
    s3, s15, s5 = math.sqrt(3.0), math.sqrt(15.0), math.sqrt(5.0)
    s358, s105 = math.sqrt(35.0 / 8.0), math.sqrt(105.0)
    s218, s7 = math.sqrt(21.0 / 8.0), math.sqrt(7.0)
    return np.stack([
        np.ones_like(x),
        s3 * y, s3 * z, s3 * x,
        s15 * x * y, s15 * y * z, 0.5 * s5 * (3 * z2 - 1), s15 * x * z,
        0.5 * s15 * (x2 - y2),
        s358 * y * (3 * x2 - y2), s105 * x * y * z, s218 * y * (5 * z2 - 1),
        0.5 * s7 * (5 * z2 - 3) * z,
        s218 * x * (5 * z2 - 1), 0.5 * s105 * (x2 - y2) * z,
        s358 * x * (x2 - y2),
    ], axis=1)


def _prep_inputs(inputs):
    inputs = {k: np.asarray(v) for k, v in inputs.items()}
    senders = inputs["senders"].astype(np.int64)
    receivers = inputs["receivers"].astype(np.int64)
    species = inputs["species"].astype(np.int64)
    vectors = inputs["vectors"].astype(np.float64)
    eps = 1.0 / math.sqrt(1.0 + float(inputs["varepsilon"])**2)
    a2 = float(inputs["alpha"])**2

    core_of = senders // NPC
    idxs = [np.nonzero(core_of == c)[0] for c in range(N_CORES)]
    maxk = max(len(i) for i in idxs)
    CAP = ((maxk + 127) // 128) * 128
    NT = CAP // 128

    # shared weights (scaled)
    sc = 1.0 / math.sqrt(N_RBF + 2*EMB)
    emb = inputs["emb"].astype(np.float64)
    W1 = inputs["W_tb1"].astype(np.float64) * sc
    tabS = emb @ W1[N_RBF:N_RBF+EMB]
    tabR = emb @ W1[N_RBF+EMB:]

    shared = {
        "wtb2": (inputs["W_tb2"] / math.sqrt(32)).astype(BF),
        "wtb3": (inputs["W_tb3"] / math.sqrt(64)).astype(BF),
        "wtb4": (inputs["W_tb4"] / math.sqrt(128)).astype(BF),
        "ww0": ((inputs["W_w0"] / math.sqrt(HIDDEN)).reshape(2, 128, 16).swapaxes(0, 1)).astype(BF),
        "ww1": ((inputs["W_w1"] * eps / math.sqrt(HIDDEN)).reshape(2, 128, 16).swapaxes(0, 1)).astype(BF),
        "ww2": ((inputs["W_w2"] * eps / math.sqrt(HIDDEN)).reshape(2, 128, 16).swapaxes(0, 1)).astype(BF),
        "wl12": ((inputs["W_l12"] / math.sqrt(HIDDEN)).reshape(2, 128, 256).swapaxes(0, 1)).astype(BF),
        "wl13": ((inputs["W_l13"] / math.sqrt(HIDDEN) * a2 / (1 + a2)).reshape(2, 128, 256).swapaxes(0, 1)).astype(BF),
        "wl22": ((inputs["W_l22"] / math.sqrt(HIDDEN)).reshape(2, 128, 256).swapaxes(0, 1)).astype(BF),
        "wl23": ((inputs["W_l23"] / math.sqrt(HIDDEN) * a2 / (1 + a2)).reshape(2, 128, 256).swapaxes(0, 1)).astype(BF),
        "wh": ((inputs["W_h"] / math.sqrt(HIDDEN)).reshape(2, 128, 128).swapaxes(0, 1)).astype(BF),
        "wout": (inputs["W_out"] / math.sqrt(128)).astype(BF),
    }
    s320 = 1.0 / math.sqrt(320)
    wl11 = inputs["W_l11"] * s320
    shared["wl11"] = wl11[:256].reshape(2, 128, 256).swapaxes(0, 1).astype(BF)
    shared["wl11t"] = wl11[256:320].astype(BF)
    wl21 = inputs["W_l21"] * s320
    shared["wl21"] = wl21[:256].reshape(2, 128, 256).swapaxes(0, 1).astype(BF)
    shared["wl21t"] = wl21[256+16:320].astype(BF)     # drop zero p0 block

    cj = _build_cj(inputs["W_v1"], inputs["W_v2"], inputs["W_v3"])
    cjd = np.zeros((128, 16, 2, 240), np.float64)
    for j in range(16):
        cjd[:, j, 0, :] = cj[j, 0:128, :]
        cjd[:, j, 1, :] = cj[j, 128:256, :]
    shared["cjd"] = cjd.astype(BF)
    r1 = _build_r1()
    r1d = np.zeros((128, 2, 64), np.float64)
    r1d[:, 0, :] = r1[0:128]; r1d[:, 1, :] = r1[128:256]
    shared["r1d"] = r1d.astype(BF)
    r2 = _build_r2()
    r2d = np.zeros((128, 2, 48), np.float64)
    r2d[:, 0, :] = r2[0:128]; r2d[0:112, 1, :] = r2[128:240]
    shared["r2d"] = r2d.astype(BF)

    repmi = np.zeros((16, 256), np.float32)
    for m in range(16):
        for i in range(16):
            repmi[i, m*16+i] = 1.0
    e16b = np.zeros((16, 256), np.float32)
    for m in range(16):
        e16b[m, m*16:(m+1)*16] = 1.0
    shared["repmi"] = repmi.astype(BF)
    shared["e16b"] = e16b.astype(BF)
    shared["ones1"] = np.ones((1, 128), np.float32).astype(BF)
    selj = np.zeros((16, 2048), np.float32)
    for j in range(16):
        selj[j, j*128:(j+1)*128] = 1.0
    shared["selj"] = selj.astype(BF)

    for k in list(shared):
        shared[k] = np.ascontiguousarray(shared[k])

    in_maps = []
    for c in range(N_CORES):
        idx = idxs[c]
        k = len(idx)
        vec = vectors[idx]
        d = np.linalg.norm(vec, axis=1)
        d = np.where(d == 0.0, 1.0, d)
        u = vec / d[:, None]
        p = 6
        env = 1.0 - 0.5*(p+1)*(p+2)*d**p + p*(p+2)*d**(p+1) - 0.5*p*(p+1)*d**(p+2)
        env = np.where(d < 1.0, env, 0.0)
        n = np.arange(1, N_RBF + 1)
        bes = (math.sqrt(2.0) * np.sin(n * math.pi * d[:, None]) / d[:, None]) \
            * env[:, None]
        Y = _sph_harm16(u)
        x1pre = bes @ W1[:N_RBF] + tabS[species[senders[idx]]] \
            + tabR[species[receivers[idx]]]

        x1full = np.zeros((CAP, 32)); x1full[:k] = x1pre
        yfull = np.zeros((CAP, 16)); yfull[:k] = Y
        envfull = np.zeros(CAP); envfull[:k] = env

        sl = senders[idx] - c * NPC
        smat = np.zeros((CAP, 256), np.float32)
        smat[np.arange(k), sl] = 1.0
        gmat = np.zeros((2, 128, CAP), np.float32)
        gmat[sl // 128, sl % 128, np.arange(k)] = 1.0

        m = dict(shared)
        m["x1pre"] = np.ascontiguousarray(x1full.T).astype(BF)
        m["ybfd"] = np.ascontiguousarray(yfull.T).astype(BF)
        m["yemd"] = np.ascontiguousarray(
            yfull.reshape(NT, 128, 16).swapaxes(0, 1)).astype(BF)
        m["envd"] = envfull[None, :].astype(BF)
        m["smat"] = np.ascontiguousarray(
            smat.reshape(NT, 128, 256)).astype(BF)
        m["gmat"] = np.ascontiguousarray(gmat.swapaxes(0, 1)).astype(BF)
        m = {k2: np.ascontiguousarray(v) for k2, v in m.items()}
        in_maps.append(m)
    return in_maps, idxs, CAP


def _run(inputs, trace=False, tmpdir=None):
    in_maps, idxs, CAP = _prep_inputs(inputs)
    if CAP not in _PROG_CACHE:
        _PROG_CACHE[CAP] = _build_program(CAP)
    nc = _PROG_CACHE[CAP]
    res = run_bass_kernel_spmd(nc, in_maps, list(range(N_CORES)), trace=trace,
                               tmpdir=tmpdir)
    out = np.zeros((E, 1), np.float32)
    for c in range(N_CORES):
        k = len(idxs[c])
        out[idxs[c], 0] = res.results[c]["outv"][0, :k]
    return out, res


def kernel(**inputs):
    out, _ = _run(inputs, trace=False)
    return out


# revision 12
# speedup vs baseline: 1.6785x; 1.0036x over previous
"""Allegro GNN on 8 TRN2 NeuronCores — Bass/Tile kernel (j-major TP rewrite).

Sharding: nodes are partitioned across cores (256 nodes/core); every edge is
routed to the core owning its *sender* node, so the scatter-sum over senders
and the gather-back are both core-local: zero collectives.

Host precomputes per-edge geometry (Y, bessel*env, env) and the two-body
first-layer pre-activation (table gathers), plus one-hot scatter/gather
matrices and the folded Clebsch-Gordan contraction matrices Cj.

Device program (feature-major activations):
  pass1: two-body MLP -> xsb; v16; scatter1 (node-major accumulate)
  pass2 per chunk: gather -> wyv; tp0 (diagonal split); j-major TP
         Vn' = sum_j Cj^T (wyv * Y_j); mlp1
  pass3: scatter2 (240-col node-major)
  pass4 per chunk: gather2 -> tp0_2 via R2; mlp2; head
"""
import math
import sys

import numpy as np

sys.path.insert(0, "/opt/trn_rl_repo")

import concourse.bacc as bacc  # noqa: E402
import concourse.mybir as mybir  # noqa: E402
from concourse import tile  # noqa: E402
from concourse.bass_utils import run_bass_kernel_spmd  # noqa: E402
import ml_dtypes  # noqa: E402

F32 = mybir.dt.float32
BF16 = mybir.dt.bfloat16
BF = ml_dtypes.bfloat16
AL = mybir.AluOpType
AF = mybir.ActivationFunctionType

E, NNODE = 32768, 2048
NUM_SPECIES, EMB = 100, 32
MUL, HIDDEN, N_RBF, LMAX = 16, 256, 8, 3
N_CORES = 8
NPC = NNODE // N_CORES          # nodes per core
SLSTART = {0: 0, 1: 1, 2: 4, 3: 9}

# ---------------------------------------------------------------- CG tensors


def _cg(j1, m1, j2, m2, j3, m3):
    if m1 + m2 != m3:
        return 0.0
    f = math.factorial
    pre = math.sqrt((2*j3+1) * f(j1+j2-j3) * f(j1-j2+j3) * f(-j1+j2+j3) / f(j1+j2+j3+1))
    pre *= math.sqrt(f(j3+m3)*f(j3-m3)*f(j1-m1)*f(j1+m1)*f(j2-m2)*f(j2+m2))
    s = 0.0
    kmin = max(0, j2 - j3 - m1, j1 - j3 + m2)
    kmax = min(j1 + j2 - j3, j1 - m1, j2 + m2)
    for k in range(kmin, kmax + 1):
        s += (-1)**k / (f(k)*f(j1+j2-j3-k)*f(j1-m1-k)*f(j2+m2-k)*f(j3-j2+m1+k)*f(j3-j1-m2+k))
    return pre * s


def _umat(l):
    U = np.zeros((2*l+1, 2*l+1), dtype=complex)
    U[l, l] = 1.0
    s2 = 1.0 / math.sqrt(2.0)
    for m in range(1, l + 1):
        U[l+m, l-m] = s2
        U[l+m, l+m] = (-1)**m * s2
        U[l-m, l-m] = 1j * s2
        U[l-m, l+m] = -1j * (-1)**m * s2
    return U


def _real_coupling(l1, l2, l3):
    C = np.zeros((2*l1+1, 2*l2+1, 2*l3+1), dtype=complex)
    for a, m1 in enumerate(range(-l1, l1+1)):
        for b, m2 in enumerate(range(-l2, l2+1)):
            for c, m3 in enumerate(range(-l3, l3+1)):
                C[a, b, c] = _cg(l1, m1, l2, m2, l3, m3)
    T = np.einsum('am,bn,ck,mnk->abc', _umat(l1), _umat(l2), _umat(l3).conj(), C)
    Tr, Ti = np.real(T), np.imag(T)
    T = Tr if np.linalg.norm(Tr) >= np.linalg.norm(Ti) else Ti
    n = np.linalg.norm(T)
    return None if n < 1e-8 else (T / n).astype(np.float32)


PATHS = {l3: [] for l3 in range(LMAX + 1)}
for _l1 in range(LMAX + 1):
    for _l2 in range(LMAX + 1):
        for _l3 in range(abs(_l1 - _l2), min(_l1 + _l2, LMAX) + 1):
            _T = _real_coupling(_l1, _l2, _l3)
            if _T is not None:
                PATHS[_l3].append((_l1, _l2, _T))
NPATH = {l3: len(PATHS[l3]) for l3 in range(LMAX + 1)}


def _build_cj(W_v1, W_v2, W_v3):
    """Cj[j, (m,i)=256, (d,k-1)=240] with T2-fold, scales, Wv baked in."""
    T2 = {l1: T[:, :, 0] for (l1, l2, T) in PATHS[0]}
    Wv = {1: W_v1.astype(np.float64), 2: W_v2.astype(np.float64),
          3: W_v3.astype(np.float64)}
    C = np.zeros((16, 256, 240), np.float64)
    for l3 in (1, 2, 3):
        scale = math.sqrt(2 * l3 + 1) / math.sqrt(MUL * NPATH[l3])
        for p, (l1, l2, T) in enumerate(PATHS[l3]):
            i0, j0 = SLSTART[l1], SLSTART[l2]
            Tf = np.einsum('kq,ijq->ijk', T2[l3].astype(np.float64),
                           T.astype(np.float64))
            # cols: (d, kabs-1); rows: (m, i); j index: j0+jj
            for k in range(2 * l3 + 1):
                kabs = SLSTART[l3] + k
                for ii in range(2 * l1 + 1):
                    for jj in range(2 * l2 + 1):
                        t = Tf[ii, jj, k] * scale
                        if t == 0.0:
                            continue
                        # C[j0+jj, m*16+i0+ii, d*15+kabs-1] += Wv[p*16+m, d]*t
                        rows = np.arange(MUL) * 16 + (i0 + ii)
                        C[j0 + jj, rows[:, None], np.arange(MUL) * 15 + (kabs - 1)] \
                            += Wv[l3][np.arange(MUL)[:, None] + p * 16,
                                      np.arange(MUL)] * t
    return C


def _build_r1():
    """R1[(m,i)=256, (p,m)=64]: tp0_1 = R1^T @ (wyv * Yrep_i)."""
    R = np.zeros((256, 64), np.float64)
    for p, (l1, l2, T) in enumerate(PATHS[0]):
        c = T[0, 0, 0] if l1 == 0 else T[1, 1, 0]
        i0 = SLSTART[l1]
        for m in range(MUL):
            for ii in range(2 * l1 + 1):
                R[m * 16 + i0 + ii, p * 16 + m] = c
    return R


def _build_r2():
    """R2[(m,k-1)=240, (l,m)=48]: tp0_2 = R2^T @ (wY2' * Vn')."""
    R = np.zeros((240, 48), np.float64)
    for li, l in enumerate((1, 2, 3)):
        for m in range(MUL):
            for k in range(2 * l + 1):
                kabs = SLSTART[l] + k
                R[m * 15 + (kabs - 1), li * 16 + m] = 1.0
    return R


# ------------------------------------------------------------- device program

_PROG_CACHE = {}


def _build_program(CAP):
    NT = CAP // 128
    CH = [(s, min(512, CAP - s)) for s in range(0, CAP, 512)]
    nc = bacc.Bacc("TRN2", target_bir_lowering=False, debug=False,
                   num_devices=N_CORES)
    D = {}

    def dp(name, shape, dt=BF16, out=False):
        D[name] = nc.declare_dram_parameter(name, list(shape), dt, isOutput=out)
        return D[name]

    dp("x1pre", [32, CAP]); dp("ybfd", [16, CAP]); dp("yemd", [128, NT, 16])
    dp("envd", [1, CAP])
    dp("smat", [NT, 128, 256]); dp("gmat", [128, 2, CAP])
    dp("cjd", [128, 16, 2, 240])
    dp("r1d", [128, 2, 64]); dp("r2d", [128, 2, 48])
    dp("repmi", [16, 256]); dp("e16b", [16, 256]); dp("ones1", [1, 128])
    dp("selj", [16, 2048])
    dp("wtb2", [32, 64]); dp("wtb3", [64, 128]); dp("wtb4", [128, 256])
    dp("ww0", [128, 2, 16]); dp("ww1", [128, 2, 16]); dp("ww2", [128, 2, 16])
    dp("wl11", [128, 2, 256]); dp("wl11t", [64, 256])
    dp("wl12", [128, 2, 256]); dp("wl13", [128, 2, 256])
    dp("wl21", [128, 2, 256]); dp("wl21t", [48, 256])
    dp("wl22", [128, 2, 256]); dp("wl23", [128, 2, 256])
    dp("wh", [128, 2, 128]); dp("wout", [128, 1])
    dp("outv", [1, CAP], dt=F32, out=True)

    with tile.TileContext(nc) as tc:
        with tc.tile_pool(name="perm", bufs=1) as perm, \
             tc.tile_pool(name="wpool", bufs=1) as wpool, \
             tc.tile_pool(name="chp", bufs=2) as chp, \
             tc.tile_pool(name="chp3", bufs=3) as chp3, \
             tc.tile_pool(name="pst", bufs=4, space="PSUM") as pst, \
             tc.tile_pool(name="pacc", bufs=1, space="PSUM") as pacc, \
             tc.tile_pool(name="psr", bufs=2, space="PSUM") as psr:

            # ---- persistent SBUF
            xsb = perm.tile([128, 2, CAP], BF16, tag="xsb", name="xsb")
            envb = perm.tile([128, CAP], BF16, tag="envb", name="envb")
            ybf = perm.tile([16, CAP], BF16, tag="ybf", name="ybf")
            yem = perm.tile([128, NT, 16], BF16, tag="yem", name="yem")
            x1s = perm.tile([32, CAP], BF16, tag="x1s", name="x1s")
            envs = perm.tile([1, CAP], BF16, tag="envs", name="envs")
            v16 = perm.tile([16, CAP], BF16, tag="v16", name="v16")
            vnpA = perm.tile([128, CAP], BF16, tag="vnpA", name="vnpA")
            vnpB = perm.tile([112, CAP], BF16, tag="vnpB", name="vnpB")
            tp0sb = perm.tile([64, CAP], BF16, tag="tp0sb", name="tp0sb")
            tp02sb = perm.tile([48, CAP], BF16, tag="tp02sb", name="tp02sb")
            node1 = perm.tile([128, 2, 256], BF16, tag="node1", name="node1")
            node2 = perm.tile([128, 2, 240], BF16, tag="node2", name="node2")
            gch = perm.tile([128, 2, CAP], BF16, tag="gch", name="gch")

            nc.sync.dma_start(ybf[:], D["ybfd"][:])
            nc.sync.dma_start(yem[:], D["yemd"][:])
            nc.sync.dma_start(x1s[:], D["x1pre"][:])
            nc.sync.dma_start(envs[:], D["envd"][:])
            nc.sync.dma_start(gch[:], D["gmat"][:])

            # ---- weights in SBUF
            W = {}
            for nm, shape in [
                ("cjd", [128, 16, 2, 240]),
                ("r1d", [128, 2, 64]), ("r2d", [128, 2, 48]),
                ("repmi", [16, 256]), ("e16b", [16, 256]), ("ones1", [1, 128]),
                ("selj", [16, 2048]),
                ("wtb2", [32, 64]), ("wtb3", [64, 128]), ("wtb4", [128, 256]),
                ("ww0", [128, 2, 16]), ("ww1", [128, 2, 16]),
                ("ww2", [128, 2, 16]),
                ("wl11", [128, 2, 256]), ("wl11t", [64, 256]),
                ("wl12", [128, 2, 256]), ("wl13", [128, 2, 256]),
                ("wl21", [128, 2, 256]), ("wl21t", [48, 256]),
                ("wl22", [128, 2, 256]), ("wl23", [128, 2, 256]),
                ("wh", [128, 2, 128]), ("wout", [128, 1]),
            ]:
                W[nm] = wpool.tile(shape, BF16, tag="w_" + nm, name="w_" + nm)
                nc.sync.dma_start(W[nm][:], D[nm][:])

            # ---- env broadcast to 128 partitions
            for (c0, cn) in CH:
                pe = pst.tile([128, 512], F32, tag="ps", name="pe")
                nc.tensor.matmul(pe[:, 0:cn], W["ones1"][:], envs[:, c0:c0+cn],
                                 start=True, stop=True)
                nc.scalar.copy(envb[:, c0:c0+cn], pe[:, 0:cn])

            # ================= pass 1: two-body MLP, v16, scatter1
            nps1 = [pacc.tile([128, 256], F32, tag=f"acc{h}", name=f"nps1_{h}")
                    for h in range(2)]
            for ci, (c0, cn) in enumerate(CH):
                x1 = chp.tile([32, 512], BF16, tag="x1", name="x1")
                nc.scalar.activation(x1[:, 0:cn], x1s[:, c0:c0+cn], AF.Silu)
                psB = pst.tile([64, 512], F32, tag="ps", name="psB")
                nc.tensor.matmul(psB[:, 0:cn], W["wtb2"][:], x1[:, 0:cn],
                                 start=True, stop=True)
                x2 = chp.tile([64, 512], BF16, tag="x2", name="x2")
                nc.scalar.activation(x2[:, 0:cn], psB[:, 0:cn], AF.Silu)
                psC = pst.tile([128, 512], F32, tag="ps", name="psC")
                nc.tensor.matmul(psC[:, 0:cn], W["wtb3"][:], x2[:, 0:cn],
                                 start=True, stop=True)
                x3 = chp.tile([128, 512], BF16, tag="x3", name="x3")
                nc.scalar.activation(x3[:, 0:cn], psC[:, 0:cn], AF.Silu)
                for oc in range(2):
                    psD = pst.tile([128, 512], F32, tag="ps", name="psD")
                    nc.tensor.matmul(psD[:, 0:cn], W["wtb4"][:, oc*128:(oc+1)*128],
                                     x3[:, 0:cn], start=True, stop=True)
                    nc.vector.tensor_tensor(xsb[:, oc, c0:c0+cn], psD[:, 0:cn],
                                            envb[:, c0:c0+cn], op=AL.mult)
                # v16
                psv = pst.tile([16, 512], F32, tag="ps", name="psv")
                nc.tensor.matmul(psv[:, 0:cn], W["ww0"][:, 0, :], xsb[:, 0, c0:c0+cn],
                                 start=True, stop=False)
                nc.tensor.matmul(psv[:, 0:cn], W["ww0"][:, 1, :], xsb[:, 1, c0:c0+cn],
                                 start=False, stop=True)
                nc.scalar.copy(v16[:, c0:c0+cn], psv[:, 0:cn])
                # scatter1 tiles of this chunk
                for t in range(c0 // 128, (c0 + cn) // 128):
                    tc_ = slice(t*128, (t+1)*128)
                    wps = psr.tile([128, 16], F32, tag="pyb", name="wps")
                    nc.tensor.matmul(wps[:], xsb[:, 0, tc_], W["ww1"][:, 0, :],
                                     start=True, stop=False)
                    nc.tensor.matmul(wps[:], xsb[:, 1, tc_], W["ww1"][:, 1, :],
                                     start=False, stop=True)
                    mbf = chp3.tile([128, 16, 16], BF16, tag="mbf", name="mbf")
                    nc.vector.tensor_tensor(
                        mbf[:], wps[:, :, None].broadcast_to((128, 16, 16)),
                        yem[:, t, None, 0:16].broadcast_to((128, 16, 16)),
                        op=AL.mult)
                    stile = chp3.tile([128, 256], BF16, tag="stile", name="stile")
                    nc.sync.dma_start(stile[:], D["smat"][t, :, :])
                    mview = mbf[:].rearrange("p a b -> p (a b)")
                    for nh in range(2):
                        nc.tensor.matmul(nps1[nh][:], stile[:, nh*128:(nh+1)*128],
                                         mview[:], start=(t == 0), stop=(t == NT-1))
            for nh in range(2):
                nc.scalar.copy(node1[:, nh, :], nps1[nh][:])

            # ================= pass 2: gather, tp0, j-major TP, mlp1
            def mlp_chunk(c0, cn, wl_a, wl_t, t_sb, tk, wl_b, wl_c):
                h1 = chp.tile([128, 2, 512], BF16, tag="h1", name="h1")
                for oc in range(2):
                    ph = pst.tile([128, 512], F32, tag="ps", name="ph")
                    ocs = slice(oc*128, (oc+1)*128)
                    nc.tensor.matmul(ph[:, 0:cn], W[wl_a][:, 0, ocs],
                                     xsb[:, 0, c0:c0+cn], start=True, stop=False)
                    nc.tensor.matmul(ph[:, 0:cn], W[wl_a][:, 1, ocs],
                                     xsb[:, 1, c0:c0+cn], start=False, stop=False)
                    nc.tensor.matmul(ph[:, 0:cn], W[wl_t][:, ocs],
                                     t_sb[0:tk, c0:c0+cn], start=False, stop=True)
                    nc.scalar.activation(h1[:, oc, 0:cn], ph[:, 0:cn], AF.Silu)
                h2 = chp.tile([128, 2, 512], BF16, tag="h2", name="h2")
                for oc in range(2):
                    ph2 = pst.tile([128, 512], F32, tag="ps", name="ph2")
                    ocs = slice(oc*128, (oc+1)*128)
                    for kc in range(2):
                        nc.tensor.matmul(ph2[:, 0:cn], W[wl_b][:, kc, ocs],
                                         h1[:, kc, 0:cn], start=(kc == 0),
                                         stop=(kc == 1))
                    nc.scalar.activation(h2[:, oc, 0:cn], ph2[:, 0:cn], AF.Silu)
                for oc in range(2):
                    ph3 = pst.tile([128, 512], F32, tag="ps", name="ph3")
                    ocs = slice(oc*128, (oc+1)*128)
                    for kc in range(2):
                        nc.tensor.matmul(ph3[:, 0:cn], W[wl_c][:, kc, ocs],
                                         h2[:, kc, 0:cn], start=(kc == 0),
                                         stop=(kc == 1))
                    ysb = chp.tile([128, 512], BF16, tag="ysb", name="ysb")
                    nc.vector.tensor_tensor(ysb[:, 0:cn], ph3[:, 0:cn],
                                            envb[:, c0:c0+cn], op=AL.mult)
                    nc.vector.scalar_tensor_tensor(
                        xsb[:, oc, c0:c0+cn], xsb[:, oc, c0:c0+cn], 0.5,
                        ysb[:, 0:cn], op0=AL.mult, op1=AL.add)

            def prep2(c0, cn):
                # gather wY (feature-major (m,i)), stage to SBUF, fold v
                pwsb = chp.tile([128, 2, 512], BF16, tag="pwsb", name="pwsb")
                for oc in range(2):
                    pw = pst.tile([128, 512], F32, tag="ps", name="pw")
                    for kc in range(2):
                        nc.tensor.matmul(pw[:, 0:cn],
                                         node1[:, kc, oc*128:(oc+1)*128],
                                         gch[:, kc, c0:c0+cn],
                                         start=(kc == 0), stop=(kc == 1))
                    nc.vector.tensor_copy(pwsb[:, oc, 0:cn], pw[:, 0:cn])
                vsb = chp.tile([128, 2, 512], BF16, tag="vsb", name="vsb")
                for oc in range(2):
                    pv = pst.tile([128, 512], F32, tag="ps", name="pv")
                    nc.tensor.matmul(pv[:, 0:cn], W["e16b"][:, oc*128:(oc+1)*128],
                                     v16[:, c0:c0+cn], start=True, stop=True)
                    nc.vector.tensor_copy(vsb[:, oc, 0:cn], pv[:, 0:cn])
                wyv = chp.tile([128, 2, 512], BF16, tag="wyv", name="wyv")
                nc.vector.tensor_tensor(wyv[:, :, 0:cn], pwsb[:, :, 0:cn],
                                        vsb[:, :, 0:cn], op=AL.mult)
                # tp0 path
                yrisb = chp.tile([128, 2, 512], BF16, tag="yrisb", name="yrisb")
                for oc in range(2):
                    pyi = pst.tile([128, 512], F32, tag="ps", name="pyi")
                    nc.tensor.matmul(pyi[:, 0:cn], W["repmi"][:, oc*128:(oc+1)*128],
                                     ybf[:, c0:c0+cn], start=True, stop=True)
                    nc.vector.tensor_copy(yrisb[:, oc, 0:cn], pyi[:, 0:cn])
                tp0p = chp.tile([128, 2, 512], BF16, tag="tp0p", name="tp0p")
                nc.vector.tensor_tensor(tp0p[:, :, 0:cn], wyv[:, :, 0:cn],
                                        yrisb[:, :, 0:cn], op=AL.mult)
                ptp0 = pst.tile([64, 512], F32, tag="ps", name="ptp0")
                for kc in range(2):
                    nc.tensor.matmul(ptp0[:, 0:cn], W["r1d"][:, kc, :],
                                     tp0p[:, kc, 0:cn], start=(kc == 0),
                                     stop=(kc == 1))
                nc.scalar.copy(tp0sb[:, c0:c0+cn], ptp0[:, 0:cn])
                return wyv

            def jloop(c0, cn, wyv):
                poutA = pacc.tile([128, 512], F32, tag="acc0", name="poutA")
                poutB = pacc.tile([112, 512], F32, tag="acc1", name="poutB")
                for j in range(16):
                    pyb = psr.tile([128, 512], F32, tag="pyb", name="pyb")
                    nc.tensor.matmul(pyb[:, 0:cn], W["selj"][:, j*128:(j+1)*128],
                                     ybf[:, c0:c0+cn], start=True, stop=True)
                    ybsb = chp3.tile([128, 512], BF16, tag="ybsb", name="ybsb")
                    nc.scalar.copy(ybsb[:, 0:cn], pyb[:, 0:cn])
                    ajs = chp3.tile([128, 2, 512], BF16, tag="ajs", name="ajs")
                    nc.vector.tensor_tensor(
                        ajs[:, :, 0:cn], wyv[:, :, 0:cn],
                        ybsb[:, None, 0:cn].broadcast_to((128, 2, cn)),
                        op=AL.mult)
                    for kc in range(2):
                        nc.tensor.matmul(poutA[:, 0:cn],
                                         W["cjd"][:, j, kc, 0:128],
                                         ajs[:, kc, 0:cn],
                                         start=(j == 0 and kc == 0),
                                         stop=(j == 15 and kc == 1))
                    for kc in range(2):
                        nc.tensor.matmul(poutB[:, 0:cn],
                                         W["cjd"][:, j, kc, 128:240],
                                         ajs[:, kc, 0:cn],
                                         start=(j == 0 and kc == 0),
                                         stop=(j == 15 and kc == 1))
                nc.scalar.copy(vnpA[:, c0:c0+cn], poutA[:, 0:cn])
                nc.scalar.copy(vnpB[:, c0:c0+cn], poutB[:, 0:cn])

            wyv_cur = prep2(*CH[0])
            for ci, (c0, cn) in enumerate(CH):
                wyv_next = prep2(*CH[ci+1]) if ci + 1 < len(CH) else None
                jloop(c0, cn, wyv_cur)
                mlp_chunk(c0, cn, "wl11", "wl11t", tp0sb, 64, "wl12", "wl13")
                wyv_cur = wyv_next

            # ================= pass 3: scatter2
            nps2 = [pacc.tile([128, 240], F32, tag=f"acc{h}", name=f"nps2_{h}")
                    for h in range(2)]
            for t in range(NT):
                tc_ = slice(t*128, (t+1)*128)
                wps2 = psr.tile([128, 16], F32, tag="pyb", name="wps2")
                nc.tensor.matmul(wps2[:], xsb[:, 0, tc_], W["ww2"][:, 0, :],
                                 start=True, stop=False)
                nc.tensor.matmul(wps2[:], xsb[:, 1, tc_], W["ww2"][:, 1, :],
                                 start=False, stop=True)
                mbf2 = chp3.tile([128, 16, 15], BF16, tag="mbf", name="mbf2")
                nc.vector.tensor_tensor(
                    mbf2[:], wps2[:, :, None].broadcast_to((128, 16, 15)),
                    yem[:, t, None, 1:16].broadcast_to((128, 16, 15)),
                    op=AL.mult)
                stile2 = chp3.tile([128, 256], BF16, tag="stile", name="stile2")
                nc.sync.dma_start(stile2[:], D["smat"][t, :, :])
                mview2 = mbf2[:].rearrange("p a b -> p (a b)")
                for nh in range(2):
                    nc.tensor.matmul(nps2[nh][:], stile2[:, nh*128:(nh+1)*128],
                                     mview2[:], start=(t == 0), stop=(t == NT-1))
            for nh in range(2):
                nc.scalar.copy(node2[:, nh, :], nps2[nh][:])

            # ================= pass 4: gather2, tp02, mlp2, head (pipelined)
            def prep4(c0, cn):
                pw2a = pst.tile([128, 512], F32, tag="ps", name="pw2a")
                pw2b = pst.tile([112, 512], F32, tag="ps", name="pw2b")
                for kc in range(2):
                    nc.tensor.matmul(pw2a[:, 0:cn], node2[:, kc, 0:128],
                                     gch[:, kc, c0:c0+cn],
                                     start=(kc == 0), stop=(kc == 1))
                for kc in range(2):
                    nc.tensor.matmul(pw2b[:, 0:cn], node2[:, kc, 128:240],
                                     gch[:, kc, c0:c0+cn],
                                     start=(kc == 0), stop=(kc == 1))
                prodsb = chp.tile([128, 2, 512], BF16, tag="prodsb", name="prodsb")
                nc.vector.tensor_tensor(prodsb[:, 0, 0:cn], pw2a[:, 0:cn],
                                        vnpA[:, c0:c0+cn], op=AL.mult)
                nc.vector.tensor_tensor(prodsb[0:112, 1, 0:cn], pw2b[:, 0:cn],
                                        vnpB[:, c0:c0+cn], op=AL.mult)
                ptp02 = pst.tile([48, 512], F32, tag="ps", name="ptp02")
                nc.tensor.matmul(ptp02[:, 0:cn], W["r2d"][:, 0, :],
                                 prodsb[:, 0, 0:cn], start=True, stop=False)
                nc.tensor.matmul(ptp02[:, 0:cn], W["r2d"][0:112, 1, :],
                                 prodsb[0:112, 1, 0:cn], start=False, stop=True)
                nc.scalar.copy(tp02sb[:, c0:c0+cn], ptp02[:, 0:cn])

            prep4(*CH[0])
            for ci, (c0, cn) in enumerate(CH):
                if ci + 1 < len(CH):
                    prep4(*CH[ci+1])
                mlp_chunk(c0, cn, "wl21", "wl21t", tp02sb, 48, "wl22", "wl23")
                # head
                psh = pst.tile([128, 512], F32, tag="ps", name="psh")
                for kc in range(2):
                    nc.tensor.matmul(psh[:, 0:cn], W["wh"][:, kc, :],
                                     xsb[:, kc, c0:c0+cn], start=(kc == 0),
                                     stop=(kc == 1))
                xh = chp.tile([128, 512], BF16, tag="xh", name="xh")
                nc.vector.tensor_tensor(xh[:, 0:cn], psh[:, 0:cn],
                                        envb[:, c0:c0+cn], op=AL.mult)
                pso = pst.tile([1, 512], F32, tag="ps", name="pso")
                nc.tensor.matmul(pso[:, 0:cn], W["wout"][:], xh[:, 0:cn],
                                 start=True, stop=True)
                osb = chp.tile([1, 512], F32, tag="osb", name="osb")
                nc.vector.tensor_copy(osb[:, 0:cn], pso[:, 0:cn])
                nc.sync.dma_start(D["outv"][:, c0:c0+cn], osb[:, 0:cn])

    nc.compile()
    return nc


# ---------------------------------------------------------------- host side


def _sph_harm16(u):
    x, y, z = u[:, 0], u[:, 1], u[:, 2]
    x2, y2, z2 = x * x, y * y, z * z
    s3, s15, s5 = math.sqrt(3.0), math.sqrt(15.0), math.sqrt(5.0)
    s358, s105 = math.sqrt(35.0 / 8.0), math.sqrt(105.0)
    s218, s7 = math.sqrt(21.0 / 8.0), math.sqrt(7.0)
    return np.stack([
        np.ones_like(x),
        s3 * y, s3 * z, s3 * x,
        s15 * x * y, s15 * y * z, 0.5 * s5 * (3 * z2 - 1), s15 * x * z,
        0.5 * s15 * (x2 - y2),
        s358 * y * (3 * x2 - y2), s105 * x * y * z, s218 * y * (5 * z2 - 1),
        0.5 * s7 * (5 * z2 - 3) * z,
        s218 * x * (5 * z2 - 1), 0.5 * s105 * (x2 - y2) * z,
        s358 * x * (x2 - y2),
    ], axis=1)


def _prep_inputs(inputs):
    inputs = {k: np.asarray(v) for k, v in inputs.items()}
    senders = inputs["senders"].astype(np.int64)
    receivers = inputs["receivers"].astype(np.int64)
    species = inputs["species"].astype(np.int64)
    vectors = inputs["vectors"].astype(np.float64)
    eps = 1.0 / math.sqrt(1.0 + float(inputs["varepsilon"])**2)
    a2 = float(inputs["alpha"])**2

    core_of = senders // NPC
    idxs = [np.nonzero(core_of == c)[0] for c in range(N_CORES)]
    maxk = max(len(i) for i in idxs)
    CAP = ((maxk + 127) // 128) * 128
    NT = CAP // 128

    # shared weights (scaled)
    sc = 1.0 / math.sqrt(N_RBF + 2*EMB)
    emb = inputs["emb"].astype(np.float64)
    W1 = inputs["W_tb1"].astype(np.float64) * sc
    tabS = emb @ W1[N_RBF:N_RBF+EMB]
    tabR = emb @ W1[N_RBF+EMB:]

    shared = {
        "wtb2": (inputs["W_tb2"] / math.sqrt(32)).astype(BF),
        "wtb3": (inputs["W_tb3"] / math.sqrt(64)).astype(BF),
        "wtb4": (inputs["W_tb4"] / math.sqrt(128)).astype(BF),
        "ww0": ((inputs["W_w0"] / math.sqrt(HIDDEN)).reshape(2, 128, 16).swapaxes(0, 1)).astype(BF),
        "ww1": ((inputs["W_w1"] * eps / math.sqrt(HIDDEN)).reshape(2, 128, 16).swapaxes(0, 1)).astype(BF),
        "ww2": ((inputs["W_w2"] * eps / math.sqrt(HIDDEN)).reshape(2, 128, 16).swapaxes(0, 1)).astype(BF),
        "wl12": ((inputs["W_l12"] / math.sqrt(HIDDEN)).reshape(2, 128, 256).swapaxes(0, 1)).astype(BF),
        "wl13": ((inputs["W_l13"] / math.sqrt(HIDDEN) * a2 / (1 + a2)).reshape(2, 128, 256).swapaxes(0, 1)).astype(BF),
        "wl22": ((inputs["W_l22"] / math.sqrt(HIDDEN)).reshape(2, 128, 256).swapaxes(0, 1)).astype(BF),
        "wl23": ((inputs["W_l23"] / math.sqrt(HIDDEN) * a2 / (1 + a2)).reshape(2, 128, 256).swapaxes(0, 1)).astype(BF),
        "wh": ((inputs["W_h"] / math.sqrt(HIDDEN)).reshape(2, 128, 128).swapaxes(0, 1)).astype(BF),
        "wout": (inputs["W_out"] / math.sqrt(128)).astype(BF),
    }
    s320 = 1.0 / math.sqrt(320)
    wl11 = inputs["W_l11"] * s320
    shared["wl11"] = wl11[:256].reshape(2, 128, 256).swapaxes(0, 1).astype(BF)
    shared["wl11t"] = wl11[256:320].astype(BF)
    wl21 = inputs["W_l21"] * s320
    shared["wl21"] = wl21[:256].reshape(2, 128, 256).swapaxes(0, 1).astype(BF)
    shared["wl21t"] = wl21[256+16:320].astype(BF)     # drop zero p0 block

    cj = _build_cj(inputs["W_v1"], inputs["W_v2"], inputs["W_v3"])
    cjd = np.zeros((128, 16, 2, 240), np.float64)
    for j in range(16):
        cjd[:, j, 0, :] = cj[j, 0:128, :]
        cjd[:, j, 1, :] = cj[j, 128:256, :]
    shared["cjd"] = cjd.astype(BF)
    r1 = _build_r1()
    r1d = np.zeros((128, 2, 64), np.float64)
    r1d[:, 0, :] = r1[0:128]; r1d[:, 1, :] = r1[128:256]
    shared["r1d"] = r1d.astype(BF)
    r2 = _build_r2()
    r2d = np.zeros((128, 2, 48), np.float64)
    r2d[:, 0, :] = r2[0:128]; r2d[0:112, 1, :] = r2[128:240]
    shared["r2d"] = r2d.astype(BF)

    repmi = np.zeros((16, 256), np.float32)
    for m in range(16):
        for i in range(16):
            repmi[i, m*16+i] = 1.0
    e16b = np.zeros((16, 256), np.float32)
    for m in range(16):
        e16b[m, m*16:(m+1)*16] = 1.0
    shared["repmi"] = repmi.astype(BF)
    shared["e16b"] = e16b.astype(BF)
    shared["ones1"] = np.ones((1, 128), np.float32).astype(BF)
    selj = np.zeros((16, 2048), np.float32)
    for j in range(16):
        selj[j, j*128:(j+1)*128] = 1.0
    shared["selj"] = selj.astype(BF)

    for k in list(shared):
        shared[k] = np.ascontiguousarray(shared[k])

    in_maps = []
    for c in range(N_CORES):
        idx = idxs[c]
        k = len(idx)
        vec = vectors[idx]
        d = np.linalg.norm(vec, axis=1)
        d = np.where(d == 0.0, 1.0, d)
        u = vec / d[:, None]
        p = 6
        env = 1.0 - 0.5*(p+1)*(p+2)*d**p + p*(p+2)*d**(p+1) - 0.5*p*(p+1)*d**(p+2)
        env = np.where(d < 1.0, env, 0.0)
        n = np.arange(1, N_RBF + 1)
        bes = (math.sqrt(2.0) * np.sin(n * math.pi * d[:, None]) / d[:, None]) \
            * env[:, None]
        Y = _sph_harm16(u)
        x1pre = bes @ W1[:N_RBF] + tabS[species[senders[idx]]] \
            + tabR[species[receivers[idx]]]

        x1full = np.zeros((CAP, 32)); x1full[:k] = x1pre
        yfull = np.zeros((CAP, 16)); yfull[:k] = Y
        envfull = np.zeros(CAP); envfull[:k] = env

        sl = senders[idx] - c * NPC
        smat = np.zeros((CAP, 256), np.float32)
        smat[np.arange(k), sl] = 1.0
        gmat = np.zeros((2, 128, CAP), np.float32)
        gmat[sl // 128, sl % 128, np.arange(k)] = 1.0

        m = dict(shared)
        m["x1pre"] = np.ascontiguousarray(x1full.T).astype(BF)
        m["ybfd"] = np.ascontiguousarray(yfull.T).astype(BF)
        m["yemd"] = np.ascontiguousarray(
            yfull.reshape(NT, 128, 16).swapaxes(0, 1)).astype(BF)
        m["envd"] = envfull[None, :].astype(BF)
        m["smat"] = np.ascontiguousarray(
            smat.reshape(NT, 128, 256)).astype(BF)
        m["gmat"] = np.ascontiguousarray(gmat.swapaxes(0, 1)).astype(BF)
        m = {k2: np.ascontiguousarray(v) for k2, v in m.items()}
        in_maps.append(m)
    return in_maps, idxs, CAP


def _run(inputs, trace=False, tmpdir=None):
    in_maps, idxs, CAP = _prep_inputs(inputs)
    if CAP not in _PROG_CACHE:
        _PROG_CACHE[CAP] = _build_program(CAP)
    nc = _PROG_CACHE[CAP]
    res = run_bass_kernel_spmd(nc, in_maps, list(range(N_CORES)), trace=trace,
                               tmpdir=tmpdir)
    out = np.zeros((E, 1), np.float32)
    for c in range(N_CORES):
        k = len(idxs[c])
        out[idxs[c], 0] = res.results[c]["outv"][0, :k]
    return out, res


def kernel(**inputs):
    out, _ = _run(inputs, trace=False)
    return out
